# revision 1
# baseline (speedup 1.0000x reference)
"""Trainium2 Bass kernel for nn_EventPairCompositionModel.

Strategy (data-parallel over batch, 8 cores, B=512 -> 64 per core):
  - Host compacts the 60MB f32 table per core to the ~24K unique rows its
    shard touches (bf16, rows padded to 384 elems = 768B), remapping all
    indices to int16.  The device then uses the fast SWDGE dma_gather
    (InstDMAGatherAnt) to fetch context/event embeddings.
  - XBAR DMA transpose (SBUF->SBUF) turns gathered bn-major rows into
    K-major tiles for the tensor engine.
  - Shared arg-composition MLP (1536->512->256, zero-padded K) in bf16.
  - Cosine numerators/denominators via small per-b matmuls that land
    n-on-partitions; norms folded together through one exp(-0.5 ln x).
  - KNRM kernel pooling via ones-matmul partition reductions, distance
    kernel path, final linear + sigmoid, all on-chip.
  - If a shard ever touches >32767 unique rows (can't happen for random
    inputs), falls back to a slow indirect-DMA gather of the full table.
All 8 cores run the identical program on their own batch shard (SPMD, no
collectives); host concatenates the 8 (64,1) outputs.
"""

import numpy as np
import ml_dtypes

import concourse.bacc as bacc
import concourse.bass as bass
import concourse.tile as tile
import concourse.mybir as mybir
from concourse.bass import IndirectOffsetOnAxis
from concourse.bass_utils import run_bass_kernel_spmd
from concourse import library_config

F32 = mybir.dt.float32
BF16 = mybir.dt.bfloat16
I16 = mybir.dt.int16
I32 = mybir.dt.int32
AF = mybir.ActivationFunctionType

# Problem shapes (hardcoded per spec)
B, N, C, E = 512, 128, 4, 300
V = 50000
H1, H2 = 512, 256
NF, NK = 8, 11
NCORES = 8
BC = B // NCORES          # 64 batches per core
EP = 384                  # padded embedding stride inside an x-row (768B)
CE = C * EP               # 1536 padded x-row length
KT = CE // 128            # 12 K-tiles for MLP1
CT = 32768                # compact table rows (int16-indexable)
GROUPS = (BC * N) // 512  # 16 groups of 512 (b,n) pairs
SUBT = 4                  # 128-bn subtiles per group
EB = 128                  # event-path width (64 real b + 64 junk)

MUS = [1.0, 0.9, 0.7, 0.5, 0.3, 0.1, -0.1, -0.3, -0.5, -0.7, -0.9]
SIGMAS = [1e-3] + [0.1] * 10

_PROGRAM_CACHE = {}


def _build_program(fast: bool):
    if fast in _PROGRAM_CACHE:
        return _PROGRAM_CACHE[fast]

    nc = bacc.Bacc("TRN2", target_bir_lowering=False, debug=False, num_swdge_queues=4)

    # ---- DRAM I/O ----
    if fast:
        ctab = nc.dram_tensor("ctab", (CT, EP), BF16, kind="ExternalInput")
        cidx = nc.dram_tensor("cidx", (128, GROUPS * 128), I16, kind="ExternalInput")
        eidx = nc.dram_tensor("eidx", (128, 32), I16, kind="ExternalInput")
    else:
        ctab = nc.dram_tensor("table", (V + 1, E), F32, kind="ExternalInput")
        cidx = nc.dram_tensor("ctxidx", (128, BC * C), I32, kind="ExternalInput")
        eidx = nc.dram_tensor("evidx", (BC, C), I32, kind="ExternalInput")
    w1t = nc.dram_tensor("w1t", (CE, H1), BF16, kind="ExternalInput")
    w2t = nc.dram_tensor("w2t", (H1, H2), BF16, kind="ExternalInput")
    wvt = nc.dram_tensor("wvt", (CE, 9), BF16, kind="ExternalInput")
    b1d = nc.dram_tensor("b1d", (128, 4), F32, kind="ExternalInput")
    b2d = nc.dram_tensor("b2d", (128, 2), F32, kind="ExternalInput")
    bvd = nc.dram_tensor("bvd", (9, 1), F32, kind="ExternalInput")
    wct = nc.dram_tensor("wct", (128, 1), F32, kind="ExternalInput")
    wckp = nc.dram_tensor("wckp", (1, NK), F32, kind="ExternalInput")
    bcd = nc.dram_tensor("bcd", (1, 1), F32, kind="ExternalInput")
    ndsq = nc.dram_tensor("ndsq", (9, BC), F32, kind="ExternalInput")
    featT = nc.dram_tensor("featT", (NF, BC), F32, kind="ExternalInput")
    out_d = nc.dram_tensor("out", (BC, 1), F32, kind="ExternalOutput")

    with tile.TileContext(nc) as tc:
        with (
            tc.tile_pool(name="consts", bufs=1) as cpool,
            tc.tile_pool(name="xg", bufs=4) as xgpool,
            tc.tile_pool(name="xt", bufs=4) as xtpool,
            tc.tile_pool(name="s1", bufs=8) as s1pool,
            tc.tile_pool(name="s2", bufs=4) as s2pool,
            tc.tile_pool(name="csq", bufs=4) as csqpool,
            tc.tile_pool(name="small", bufs=2) as smpool,
            tc.tile_pool(name="pm1", bufs=2, space="PSUM") as pm1,
            tc.tile_pool(name="pm2", bufs=2, space="PSUM") as pm2,
            tc.tile_pool(name="ptn", bufs=1, space="PSUM") as ptn,
            tc.tile_pool(name="pmisc", bufs=2, space="PSUM") as pmisc,
        ):
            # ---- load constants ----
            if fast:
                nc.gpsimd.load_library(library_config.mlp)
                cidx_s = cpool.tile([128, GROUPS * 128], I16)
                nc.sync.dma_start(cidx_s[:], cidx.ap())
                eidx_s = cpool.tile([128, 32], I16)
                nc.sync.dma_start(eidx_s[:], eidx.ap())
            w1t_s = cpool.tile([128, KT * H1], BF16)
            nc.sync.dma_start(
                w1t_s[:].rearrange("p (t m) -> p t m", t=KT),
                w1t.ap().rearrange("(t p) m -> p t m", p=128),
            )
            w2t_s = cpool.tile([128, 4 * H2], BF16)
            nc.scalar.dma_start(
                w2t_s[:].rearrange("p (t m) -> p t m", t=4),
                w2t.ap().rearrange("(t p) m -> p t m", p=128),
            )
            wvt_s = cpool.tile([128, KT * 9], BF16)
            nc.scalar.dma_start(
                wvt_s[:].rearrange("p (t m) -> p t m", t=KT),
                wvt.ap().rearrange("(t p) m -> p t m", p=128),
            )
            b1_s = cpool.tile([128, 4], F32)
            nc.sync.dma_start(b1_s[:], b1d.ap())
            b2_s = cpool.tile([128, 2], F32)
            nc.sync.dma_start(b2_s[:], b2d.ap())
            bv_s = cpool.tile([9, 1], F32)
            nc.sync.dma_start(bv_s[:], bvd.ap())
            wct_s = cpool.tile([128, 1], F32)
            nc.sync.dma_start(wct_s[:], wct.ap())
            wckp_s = cpool.tile([1, NK], F32)
            nc.sync.dma_start(wckp_s[:], wckp.ap())
            bc_s = cpool.tile([1, 1], F32)
            nc.sync.dma_start(bc_s[:], bcd.ap())
            if not fast:
                cidx_s = cpool.tile([128, BC * C], I32)
                nc.sync.dma_start(cidx_s[:], cidx.ap())
                eidx_s = cpool.tile([BC, C], I32)
                nc.sync.dma_start(eidx_s[:], eidx.ap())
            ndsq_s = cpool.tile([9, BC], F32)
            nc.sync.dma_start(ndsq_s[:], ndsq.ap())
            feat_s = cpool.tile([128, BC], F32)
            nc.vector.memset(feat_s[:], 0.0)
            nc.sync.dma_start(feat_s[64 : 64 + NF, :], featT.ap())
            ones_s = cpool.tile([128, 1], BF16)
            nc.vector.memset(ones_s[:], 1.0)
            onesrow_s = cpool.tile([1, 128], F32)
            nc.vector.memset(onesrow_s[:], 1.0)
            onesf_s = cpool.tile([128, 1], F32)
            nc.vector.memset(onesf_s[:], 1.0)
            eps_s = cpool.tile([128, 1], F32)
            nc.vector.memset(eps_s[:], 1e-20)
            mub_s = cpool.tile([128, NK], F32)
            for k in range(NK):
                nc.vector.memset(mub_s[:, k : k + 1], -MUS[k])

            # ---- event path (EB=128 lanes, only 0..63 meaningful) ----
            xeT = cpool.tile([128, KT * EB], BF16)
            if fast:
                # transpose-mode gather lands K-major directly:
                # xeT[p, jj, c*128+b] = emb_{b,c}[jj*128+p]
                nc.gpsimd.dma_gather(
                    out_ap=xeT[:].rearrange("p (j i) -> p j i", j=3),
                    in_ap=ctab.ap(),
                    idxs_ap=eidx_s[:],
                    num_idxs=512,
                    num_idxs_reg=512,
                    elem_size=EP,
                    transpose=True,
                )
            else:
                xe = cpool.tile([EB, CE], BF16)
                nc.vector.memset(xe[:], 0.0)
                nc.gpsimd.indirect_dma_start(
                    out=xe[0:BC, :].rearrange("p (c e) -> p c e", c=C)[:, :, 0:E],
                    out_offset=None,
                    in_=ctab.ap(),
                    in_offset=IndirectOffsetOnAxis(ap=eidx_s[:], axis=0),
                )
                nc.sync.dma_start_transpose(
                    xeT[:].rearrange("p (j i) -> p j i", j=KT), xe[:]
                )

            def xeT_k(j):
                # K-tile j = 3*c + jj of the event activations
                if fast:
                    return xeT[:, 512 * (j % 3) + 128 * (j // 3) :][:, 0:128]
                return xeT[:, EB * j : EB * (j + 1)]

            s1e = cpool.tile([128, 4 * EB], BF16)
            for m in range(4):
                pe = pmisc.tile([128, EB], F32, tag="pmisc", name="pe")
                for j in range(KT):
                    nc.tensor.matmul(
                        pe[:],
                        w1t_s[:, H1 * j + 128 * m : H1 * j + 128 * m + 128],
                        xeT_k(j),
                        start=(j == 0),
                        stop=(j == KT - 1),
                    )
                nc.scalar.activation(
                    s1e[:, EB * m : EB * (m + 1)], pe[:], AF.Relu,
                    bias=b1_s[:, m : m + 1],
                )

            eh2 = [
                cpool.tile([128, EB], BF16, tag=f"eh2_{k}", name=f"eh2_{k}")
                for k in range(2)
            ]
            for m in range(2):
                pe2 = pmisc.tile([128, EB], F32, tag="pmisc", name="pe2")
                for j in range(4):
                    nc.tensor.matmul(
                        pe2[:],
                        w2t_s[:, H2 * j + 128 * m : H2 * j + 128 * m + 128],
                        s1e[:, EB * j : EB * (j + 1)],
                        start=(j == 0),
                        stop=(j == 3),
                    )
                nc.scalar.activation(
                    eh2[m][:], pe2[:], AF.Relu, bias=b2_s[:, m : m + 1]
                )

            # variances -> dist_emb rows 32..40 of feat_s
            pv = pmisc.tile([9, EB], F32, tag="pmisc", name="pv")
            for j in range(KT):
                nc.tensor.matmul(
                    pv[:],
                    wvt_s[:, 9 * j : 9 * (j + 1)],
                    xeT_k(j),
                    start=(j == 0),
                    stop=(j == KT - 1),
                )
            ez_s = smpool.tile([9, EB], F32)
            nc.scalar.activation(ez_s[:], pv[:], AF.Exp, bias=bv_s[:])
            ez1_s = smpool.tile([9, EB], F32)
            nc.vector.tensor_scalar_add(ez1_s[:], ez_s[:], 1.0)
            var_s = smpool.tile([9, EB], F32)
            nc.scalar.activation(var_s[:], ez1_s[:], AF.Ln)
            rv_s = smpool.tile([9, EB], F32)
            nc.vector.reciprocal(rv_s[:], var_s[:])
            q_s = smpool.tile([9, BC], F32)
            nc.vector.tensor_mul(q_s[:], ndsq_s[:], rv_s[:, 0:BC])
            nc.scalar.activation(feat_s[32:41, :], q_s[:], AF.Exp)

            # |e|^2 per b, broadcast to all 128 partitions via outer product
            esq = [
                smpool.tile([128, EB], BF16, tag=f"esq_{k}", name=f"esq_{k}")
                for k in range(2)
            ]
            for k in range(2):
                nc.vector.tensor_mul(esq[k][:], eh2[k][:], eh2[k][:])
            pne = pmisc.tile([1, EB], F32, tag="pmisc", name="pne")
            for k in range(2):
                nc.tensor.matmul(
                    pne[:], ones_s[:], esq[k][:], start=(k == 0), stop=(k == 1)
                )
            ne2_s = smpool.tile([1, BC], F32)
            nc.scalar.copy(ne2_s[:], pne[:, 0:BC])
            pne2bc = pmisc.tile([128, BC], F32, tag="pmisc", name="pne2bc")
            nc.tensor.matmul(
                pne2bc[:], onesrow_s[:], ne2_s[:], start=True, stop=True
            )
            ne2bc_s = cpool.tile([128, BC], F32)
            nc.scalar.copy(ne2bc_s[:], pne2bc[:])

            # persistent SBUF accumulators, n on partitions, b on free
            traw_s = cpool.tile([128, BC], F32)
            ncsq_s = cpool.tile([128, BC], F32)

            # ---- context groups ----
            for g in range(GROUPS):
                xt = xtpool.tile([128, KT * 512], BF16)
                if fast:
                    # per subtile s: xt[p, s, jj, c*128+pbn] (s-major blocks)
                    for s in range(SUBT):
                        nc.gpsimd.dma_gather(
                            out_ap=xt[:]
                            .rearrange("p (z j i) -> p z j i", z=SUBT, j=3)[
                                :, s, :, :
                            ],
                            in_ap=ctab.ap(),
                            idxs_ap=cidx_s[
                                :, 32 * (SUBT * g + s) : 32 * (SUBT * g + s + 1)
                            ],
                            num_idxs=512,
                            num_idxs_reg=512,
                            elem_size=EP,
                            transpose=True,
                        )
                else:
                    xg = xgpool.tile([128, SUBT * CE], BF16)
                    nc.vector.memset(
                        xg[:].rearrange("p (q e) -> p q e", e=EP)[:, :, E:EP],
                        0.0,
                    )
                    for s in range(SUBT):
                        nc.gpsimd.indirect_dma_start(
                            out=xg[:]
                            .rearrange("p (q c e) -> p q c e", q=SUBT, c=C)[
                                :, s, :, 0:E
                            ],
                            out_offset=None,
                            in_=ctab.ap(),
                            in_offset=IndirectOffsetOnAxis(
                                ap=cidx_s[
                                    :, (SUBT * g + s) * C : (SUBT * g + s + 1) * C
                                ],
                                axis=0,
                            ),
                        )
                    for s in range(SUBT):
                        nc.sync.dma_start_transpose(
                            xt[:].rearrange(
                                "p (j z i) -> p j z i", j=KT, z=SUBT
                            )[:, :, s, :],
                            xg[:, CE * s : CE * (s + 1)],
                        )

                def xt_k(j):
                    # K-tile j = 3*c + jj; cols ordered (s, pbn)
                    if fast:
                        off = 512 * (j % 3) + 128 * (j // 3)
                        return xt[:].rearrange(
                            "p (z x) -> p z x", z=SUBT
                        )[:, :, off : off + 128]
                    return xt[:, 512 * j : 512 * (j + 1)]

                s1 = [
                    s1pool.tile([128, 512], BF16, tag=f"s1_{m}", name=f"s1_{m}")
                    for m in range(4)
                ]
                for m in range(4):
                    p1 = pm1.tile([128, 512], F32)
                    for j in range(KT):
                        nc.tensor.matmul(
                            p1[:],
                            w1t_s[:, H1 * j + 128 * m : H1 * j + 128 * m + 128],
                            xt_k(j),
                            start=(j == 0),
                            stop=(j == KT - 1),
                        )
                    nc.scalar.activation(
                        s1[m][:], p1[:], AF.Relu, bias=b1_s[:, m : m + 1]
                    )

                s2 = [
                    s2pool.tile([128, 512], BF16, tag=f"s2_{m}", name=f"s2_{m}")
                    for m in range(2)
                ]
                for m in range(2):
                    p2 = pm2.tile([128, 512], F32)
                    for j in range(4):
                        nc.tensor.matmul(
                            p2[:],
                            w2t_s[:, H2 * j + 128 * m : H2 * j + 128 * m + 128],
                            s1[j][:],
                            start=(j == 0),
                            stop=(j == 3),
                        )
                    nc.scalar.activation(
                        s2[m][:], p2[:], AF.Relu, bias=b2_s[:, m : m + 1]
                    )

                csq = [
                    csqpool.tile([128, 512], BF16, tag=f"csq_{m}", name=f"csq_{m}")
                    for m in range(2)
                ]
                for m in range(2):
                    nc.vector.tensor_mul(csq[m][:], s2[m][:], s2[m][:])

                # raw dots and |c|^2, n on partitions, one column per b
                pT = ptn.tile([128, SUBT], F32, tag="pT", name="pT")
                pN = ptn.tile([128, SUBT], F32, tag="pN", name="pN")
                for s in range(SUBT):
                    b = SUBT * g + s
                    for k in range(2):
                        nc.tensor.matmul(
                            pT[:, s : s + 1],
                            s2[k][:, 128 * s : 128 * (s + 1)],
                            eh2[k][:, b : b + 1],
                            start=(k == 0),
                            stop=(k == 1),
                        )
                    for k in range(2):
                        nc.tensor.matmul(
                            pN[:, s : s + 1],
                            csq[k][:, 128 * s : 128 * (s + 1)],
                            ones_s[:],
                            start=(k == 0),
                            stop=(k == 1),
                        )
                nc.scalar.copy(traw_s[:, SUBT * g : SUBT * (g + 1)], pT[:])
                nc.scalar.copy(ncsq_s[:, SUBT * g : SUBT * (g + 1)], pN[:])

            # ---- kernel pooling (tiles are [n=128, b=64]) ----
            prodn = smpool.tile([128, BC], F32, tag="prodn")
            nc.vector.tensor_mul(prodn[:], ncsq_s[:], ne2bc_s[:])
            lnp = smpool.tile([128, BC], F32, tag="lnp")
            nc.scalar.activation(lnp[:], prodn[:], AF.Ln, bias=eps_s[:])
            nrmf = smpool.tile([128, BC], F32, tag="nrmf")
            nc.scalar.activation(nrmf[:], lnp[:], AF.Exp, scale=-0.5)
            trans = cpool.tile([128, BC], F32)
            nc.vector.tensor_mul(trans[:], traw_s[:], nrmf[:])

            kpp_s = cpool.tile([1, NK * BC], F32)
            for k in range(NK):
                sq = smpool.tile([128, BC], F32, tag="sq", name="sq")
                nc.scalar.activation(
                    sq[:], trans[:], AF.Square, bias=mub_s[:, k : k + 1]
                )
                arg = smpool.tile([128, BC], F32, tag="arg", name="arg")
                nc.vector.tensor_scalar(
                    arg[:], sq[:],
                    -1.0 / (2.0 * SIGMAS[k] ** 2), -87.0,
                    mybir.AluOpType.mult, mybir.AluOpType.max,
                )
                ek = smpool.tile([128, BC], F32, tag="ek", name="ek")
                nc.scalar.activation(ek[:], arg[:], AF.Exp)
                pp = pmisc.tile([1, BC], F32, tag="pmisc", name="pp")
                nc.tensor.matmul(pp[:], onesf_s[:], ek[:], start=True, stop=True)
                nc.scalar.copy(kpp_s[:, BC * k : BC * (k + 1)], pp[:])

            kpc_s = smpool.tile([1, NK * BC], F32, tag="kpc")
            nc.vector.tensor_scalar_max(kpc_s[:], kpp_s[:], 1e-10)
            kpl_s = smpool.tile([1, NK * BC], F32, tag="kpl")
            nc.scalar.activation(kpl_s[:], kpc_s[:], AF.Ln)

            # weighted sum over k: kps[b] = sum_k wckp[k] * kpl[k, b]
            kpw_s = smpool.tile([1, BC * NK], F32, tag="kpw")
            kpl_v = kpl_s[:].rearrange("p (k b) -> p b k", k=NK)
            wck_v = wckp_s[:][:, None, :].broadcast_to([1, BC, NK])
            kpw_v = kpw_s[:].rearrange("p (b k) -> p b k", b=BC)
            nc.vector.tensor_tensor(
                out=kpw_v, in0=kpl_v, in1=wck_v, op=mybir.AluOpType.mult
            )
            kps_s = smpool.tile([1, BC], F32, tag="kps")
            nc.vector.reduce_sum(
                out=kps_s[:], in_=kpw_v, axis=mybir.AxisListType.X
            )

            # ---- final score ----
            psc = pmisc.tile([1, BC], F32, tag="pmisc", name="psc")
            nc.tensor.matmul(psc[:], wct_s[:], feat_s[:], start=True, stop=True)
            tot_s = smpool.tile([1, BC], F32, tag="tot")
            nc.vector.tensor_add(tot_s[:], psc[:], kps_s[:])
            emx = smpool.tile([1, BC], F32, tag="emx")
            nc.scalar.activation(emx[:], tot_s[:], AF.Exp, bias=bc_s[:], scale=-1.0)
            emx1 = smpool.tile([1, BC], F32, tag="emx1")
            nc.vector.tensor_scalar_add(emx1[:], emx[:], 1.0)
            outs = smpool.tile([1, BC], F32, tag="outs")
            nc.vector.reciprocal(outs[:], emx1[:])
            nc.sync.dma_start(out_d.ap().rearrange("b one -> one b"), outs[:])

    nc.compile()

    # Spread SWDGE gathers across the 4 queues. The ucode locks each DMASW
    # semaphore lane to one queue, and Tile assigns lanes round-robin in
    # scheduled order, so derive queue from the assigned lane post-compile.
    import re as _re
    for blk in nc.m.functions[0].blocks:
        for inst in blk.instructions:
            if type(inst).__name__ == "InstDMAGatherAnt":
                for u in inst.sync_info.on_update:
                    m = _re.match(r"DMASW(\d+)_", u.ant_name or "")
                    if m:
                        inst.queue_num = int(m.group(1)) % 4
                        break

    _PROGRAM_CACHE[fast] = nc
    return nc


def _wrap16(flat_idx):
    """int16 index list -> (128, n/16) tile layout: unwrapped[i] =
    tile[i % 16, i // 16], replicated into all 8 16-partition stripes."""
    n = flat_idx.shape[0]
    t = np.zeros((16, n // 16), np.int16)
    t[np.arange(n) % 16, np.arange(n) // 16] = flat_idx
    return np.tile(t, (8, 1))


def _prep_core_inputs(inputs, core, fast):
    """Host-side shard + weight re-layouts for one core."""
    W1 = np.asarray(inputs["W1"], np.float32)
    W2 = np.asarray(inputs["W2"], np.float32)
    Wv = np.asarray(inputs["Wv"], np.float32)
    Wc = np.asarray(inputs["Wc"], np.float32)
    b1 = np.asarray(inputs["b1"], np.float32)
    b2 = np.asarray(inputs["b2"], np.float32)
    bv = np.asarray(inputs["bv"], np.float32)
    bc = np.asarray(inputs["bc"], np.float32)

    sl = slice(core * BC, (core + 1) * BC)
    ev = np.asarray(inputs["batch_event"][sl], np.int64)          # (BC, C)
    feats = np.asarray(inputs["batch_features"][sl], np.float32)  # (BC, NF)
    dists = np.asarray(inputs["batch_distances"][sl], np.float32) # (BC, 9)
    ctx = np.asarray(inputs["batch_context"][sl], np.int64)       # (BC, N, C)

    bf = ml_dtypes.bfloat16
    # W1.T with K padded 300->EP per component, zeros in the pad rows
    w1t = np.zeros((CE, H1), np.float32)
    for c in range(C):
        w1t[EP * c : EP * c + E, :] = W1[:, E * c : E * (c + 1)].T
    wvt = np.zeros((CE, 9), np.float32)
    wvt[EP * 1 : EP * 1 + E, :] = Wv.T  # predicates = component 1

    wc_full = np.zeros((128,), np.float32)
    wc_full[32 : 32 + 9] = Wc[0, 0:9]          # dist_emb block
    wc_full[64 : 64 + NF] = Wc[0, 9 : 9 + NF]  # batch_features block
    wckp = (Wc[0, NF + 9 :] * 0.01).astype(np.float32)  # kp block, 0.01 folded

    m = {
        "w1t": w1t.astype(bf),
        "w2t": np.ascontiguousarray(W2.T).astype(bf),
        "wvt": wvt.astype(bf),
        "b1d": np.ascontiguousarray(b1.reshape(4, 128).T),
        "b2d": np.ascontiguousarray(b2.reshape(2, 128).T),
        "bvd": bv.reshape(9, 1),
        "wct": wc_full.reshape(-1, 1),
        "wckp": wckp.reshape(1, NK),
        "bcd": -bc.reshape(1, 1),
        "ndsq": np.ascontiguousarray(-(dists * dists).T),
        "featT": np.ascontiguousarray(feats.T),
    }

    if fast:
        table = np.asarray(inputs["event_table"])
        allidx = np.concatenate([ctx.reshape(-1), ev.reshape(-1)])
        uniq, inv = np.unique(allidx, return_inverse=True)
        assert len(uniq) <= CT
        ctab = np.zeros((CT, EP), bf)
        ctab[: len(uniq), :E] = np.asarray(table[uniq], np.float32)
        rctx = inv[: ctx.size].astype(np.int16).reshape(BC, N, C)
        rev = inv[ctx.size :].astype(np.int16).reshape(BC, C)

        # context: per (g, s) gather of 512 idx with i = c*128 + p
        ci = rctx.reshape(GROUPS, SUBT, N, C).transpose(0, 1, 3, 2)  # g,s,c,p
        cidx = np.concatenate(
            [
                _wrap16(ci[g, s].reshape(-1))
                for g in range(GROUPS)
                for s in range(SUBT)
            ],
            axis=1,
        )
        # event: i = c*128 + b; b >= BC -> row 0 junk
        ei = np.zeros((C, 128), np.int16)
        ei[:, :BC] = rev.T
        m["ctab"] = ctab
        m["cidx"] = np.ascontiguousarray(cidx)
        m["eidx"] = np.ascontiguousarray(_wrap16(ei.reshape(-1)))
    else:
        m["table"] = np.ascontiguousarray(
            np.asarray(inputs["event_table"], np.float32)
        )
        m["ctxidx"] = np.ascontiguousarray(
            ctx.astype(np.int32).transpose(1, 0, 2).reshape(128, BC * C)
        )
        m["evidx"] = ev.astype(np.int32)
    return m


def kernel(**inputs) -> np.ndarray:
    # fast path requires every shard's unique row count to fit int16
    fast = True
    ctx = np.asarray(inputs["batch_context"], np.int64)
    ev = np.asarray(inputs["batch_event"], np.int64)
    for core in range(NCORES):
        sl = slice(core * BC, (core + 1) * BC)
        nuniq = len(np.unique(np.concatenate(
            [ctx[sl].reshape(-1), ev[sl].reshape(-1)])))
        if nuniq > CT:
            fast = False
            break
    nc = _build_program(fast)
    in_maps = [_prep_core_inputs(inputs, core, fast) for core in range(NCORES)]
    res = run_bass_kernel_spmd(nc, in_maps, core_ids=list(range(NCORES)))
    return np.concatenate([r["out"] for r in res.results], axis=0)


if __name__ == "__main__":
    nc = _build_program(True)
    print("program built ok")



# revision 21
# speedup vs baseline: 1.2574x; 1.2574x over previous
"""Trainium2 Bass kernel for nn_EventPairCompositionModel (fp8 DoubleRow).

Strategy (data-parallel over batch, 8 cores, B=512 -> 64 per core):
  - Host compacts the 60MB f32 table per core to the ~24K unique rows its
    shard touches, stored as fp8e4m3 (x8 scale) rows padded to 512 bytes and
    DECLARED int16 (256 units) so the SWDGE transpose-gather's 16-bit
    granularity lands fp8 element pairs (2u, 2u+1) on partition u%128 --
    exactly the [K, 2, N] layout MatmulPerfMode.DoubleRow wants.
  - Context MLP (1200->512->256) entirely in fp8 DoubleRow (2 K-rows/cycle).
    One 2048-idx gather per group of 512 (b,n) pairs; weights stationary
    reused across 2 groups per pass, redundant LDWEIGHTS removed by a
    post-compile pass so the PE streams matmuls at the DR rate.
  - The 64 event tuples ride pass 0 as a narrow (64-col) third group.
  - Cosine numerators via one fp8 DR "gram" matmul per group (event reprs
    stationary), |c|^2 via a DR ones-matmul over squared activations; diag
    blocks extracted into a [b, n] layout; KNRM pooling on [64,128] tiles.
  - Distance-kernel path in bf16 off the event gather; final linear+sigmoid
    in [64 batch partitions, feature] layout.
All 8 cores run the identical program on their own batch shard (SPMD).
Host falls back to a numpy reference if a shard exceeds the int16 compact
table (cannot happen for the random fill).
"""

import numpy as np
import ml_dtypes

import concourse.bacc as bacc
import concourse.bass as bass
import concourse.tile as tile
import concourse.mybir as mybir
from concourse.bass_utils import run_bass_kernel_spmd
from concourse import library_config

F32 = mybir.dt.float32
BF16 = mybir.dt.bfloat16
FP8 = mybir.dt.float8e4
I16 = mybir.dt.int16
AF = mybir.ActivationFunctionType
DR = mybir.MatmulPerfMode.DoubleRow

# Problem shapes (hardcoded per spec)
B, N, C, E = 512, 128, 4, 300
V = 50000
H1, H2 = 512, 256
NF, NK = 8, 11
NCORES = 8
BC = B // NCORES          # 64 batches per core
CT = 32768                # compact table rows (int16-indexable)
RU = 256                  # int16 units per table row (512 fp8 elems)
GROUPS = (BC * N) // 512  # 16 groups of 512 (b,n) pairs
NKP = 8                   # MLP1 K-pairs per m-tile (4 comps x 2 slots)
FD = 28                   # features: 9 dist + 8 extracted + 11 kp

MUS = [1.0, 0.9, 0.7, 0.5, 0.3, 0.1, -0.1, -0.3, -0.5, -0.7, -0.9]
SIGMAS = [1e-3] + [0.1] * 10

_PROGRAM_CACHE = {}


def _build_program(fast: bool = True):
    if True in _PROGRAM_CACHE:
        return _PROGRAM_CACHE[True]

    nc = bacc.Bacc("TRN2", target_bir_lowering=False, debug=False, num_swdge_queues=4)

    # ---- DRAM I/O ----
    ctab = nc.dram_tensor("ctab", (CT, RU), I16, kind="ExternalInput")
    cidx = nc.dram_tensor("cidx", (128, GROUPS * 128), I16, kind="ExternalInput")
    eidx = nc.dram_tensor("eidx", (128, 32), I16, kind="ExternalInput")
    w1p = nc.dram_tensor("w1p", (128, NKP * 4 * 256), FP8, kind="ExternalInput")
    w2p = nc.dram_tensor("w2p", (128, 2 * 2 * 256), FP8, kind="ExternalInput")
    wvp = nc.dram_tensor("wvp", (128, 4 * 9), BF16, kind="ExternalInput")
    b1d = nc.dram_tensor("b1d", (128, 4), F32, kind="ExternalInput")
    b2d = nc.dram_tensor("b2d", (128, 2), F32, kind="ExternalInput")
    ebv = nc.dram_tensor("ebv", (BC, 9), F32, kind="ExternalInput")
    ndsq = nc.dram_tensor("ndsq", (BC, 9), F32, kind="ExternalInput")
    featd = nc.dram_tensor("featd", (BC, NF), F32, kind="ExternalInput")
    wcr = nc.dram_tensor("wcr", (BC, FD), F32, kind="ExternalInput")
    bcr = nc.dram_tensor("bcr", (BC, 1), F32, kind="ExternalInput")
    out_d = nc.dram_tensor("out", (BC, 1), F32, kind="ExternalOutput")

    with tile.TileContext(nc) as tc:
        with (
            tc.tile_pool(name="consts", bufs=1) as cpool,
            tc.tile_pool(name="xt", bufs=4) as xtpool,
            tc.tile_pool(name="s1", bufs=3) as s1pool,
            tc.tile_pool(name="s2", bufs=3) as s2pool,
            tc.tile_pool(name="csq", bufs=3) as csqpool,
            tc.tile_pool(name="small", bufs=2) as smpool,
            tc.tile_pool(name="pm1", bufs=3, space="PSUM") as pm1,
            tc.tile_pool(name="pm2", bufs=2, space="PSUM") as pm2,
            tc.tile_pool(name="pg", bufs=2, space="PSUM") as pgpool,
            tc.tile_pool(name="pmisc", bufs=1, space="PSUM") as pmisc,
        ):
            nc.gpsimd.load_library(library_config.mlp)

            # ---- load constants ----
            cidx_s = cpool.tile([128, GROUPS * 128], I16)
            nc.sync.dma_start(cidx_s[:], cidx.ap())
            eidx_s = cpool.tile([128, 32], I16)
            nc.sync.dma_start(eidx_s[:], eidx.ap())
            w1p_s = cpool.tile([128, NKP * 4 * 256], FP8)
            nc.sync.dma_start(w1p_s[:], w1p.ap())
            w2p_s = cpool.tile([128, 2 * 2 * 256], FP8)
            nc.scalar.dma_start(w2p_s[:], w2p.ap())
            wvp_s = cpool.tile([128, 4 * 9], BF16)
            nc.scalar.dma_start(wvp_s[:], wvp.ap())
            b1_s = cpool.tile([128, 4], F32)
            nc.sync.dma_start(b1_s[:], b1d.ap())
            b2_s = cpool.tile([128, 2], F32)
            nc.sync.dma_start(b2_s[:], b2d.ap())
            ebv_s = cpool.tile([BC, 9], F32)
            nc.scalar.dma_start(ebv_s[:], ebv.ap())
            ndsq_s = cpool.tile([BC, 9], F32)
            nc.scalar.dma_start(ndsq_s[:], ndsq.ap())
            wcr_s = cpool.tile([BC, FD], F32)
            nc.scalar.dma_start(wcr_s[:], wcr.ap())
            bcr_s = cpool.tile([BC, 1], F32)
            nc.scalar.dma_start(bcr_s[:], bcr.ap())
            F_s = cpool.tile([BC, FD], F32)
            nc.sync.dma_start(F_s[:, 9 : 9 + NF], featd.ap())

            # ones pair for |c|^2 matmuls; K-group dim at stride 16 to satisfy
            # the dual-fp8 LDWEIGHTS AP restriction (pair step % 16 == 0)
            ones2c_s = cpool.tile([128, 32], FP8)
            nc.vector.memset(ones2c_s[:], 1.0)
            eps_s = cpool.tile([BC, 1], F32)
            nc.vector.memset(eps_s[:], 1e-30)
            mub_s = cpool.tile([BC, NK], F32)
            for k in range(NK):
                nc.vector.memset(mub_s[:, k : k + 1], -MUS[k])

            # persistent accumulators
            traw_s = cpool.tile([BC, 128], F32)        # 16*dot per (b, n)
            drow_s = cpool.tile([1, 512 * GROUPS], F32)  # dots, (g,s,n) on part 0
            nrow_s = cpool.tile([1, 512 * GROUPS], F32)  # |c|^2 likewise
            sgram_s = cpool.tile([128, 2 * BC], FP8)   # event reprs [p, m(2), 64]
            cse_s = cpool.tile([128, 2 * BC], FP8)     # their squares
            s1e_s = cpool.tile([128, 4 * BC], FP8)     # event s1 [p, mj(4), 64]
            predb_s = cpool.tile([128, 4 * BC], BF16)  # predicates [p, f(4), 64]
            ne2_s = cpool.tile([BC, 1], F32)
            ncsq0_s = cpool.tile([BC, 128], F32)
            trans_s = cpool.tile([BC, 128], F32)
            pooled_s = cpool.tile([BC, NK], F32)

            # ---- gathers (issued lazily so pool-slot reuse stays WAR-safe) ----
            xe_s = cpool.tile([128, 2 * 512], I16)   # event gather
            nc.gpsimd.dma_gather(
                out_ap=xe_s[:].rearrange("p (s i) -> p s i", s=2),
                in_ap=ctab.ap(),
                idxs_ap=eidx_s[:],
                num_idxs=512,
                num_idxs_reg=512,
                elem_size=RU,
                transpose=True,
            )
            xts = {}

            def issue_gather(g):
                # one 512-idx gather per component (512-idx bursts fit the
                # SWDGE descriptor-ring carveout; 2048 would deadlock it)
                if g >= GROUPS:
                    return
                xt = xtpool.tile([128, 4 * 2 * 512], I16, tag="xt", name=f"xt{g}")
                xv = xt[:].rearrange("p (c s r) -> p c s r", c=4, s=2)
                for c in range(C):
                    nc.gpsimd.dma_gather(
                        out_ap=xv[:, c, :, :],
                        in_ap=ctab.ap(),
                        idxs_ap=cidx_s[:, 32 * (4 * g + c) : 32 * (4 * g + c + 1)],
                        num_idxs=512,
                        num_idxs_reg=512,
                        elem_size=RU,
                        transpose=True,
                    )
                xts[g] = xt

            for g in range(4):
                issue_gather(g)

            def ctx_rhs(g, c, s, w):
                # [p, 2(byte), w cols] fp8 view of gathered group g, comp c, slot s
                v = xts[g][:].bitcast(FP8).rearrange(
                    "p (c s r i) -> p c s i r", c=4, s=2, i=2
                )
                return v[:, c, s, :, 0:w]

            def evt_rhs(c, s):
                v = xe_s[:].bitcast(FP8).rearrange("p (s r i) -> p s i r", s=2, i=2)
                return v[:, s, :, 128 * c : 128 * c + BC]

            def w1_ap(kp, m):
                return w1p_s[:].rearrange(
                    "p (kp m i c) -> p kp m i c", kp=NKP, m=4, i=2
                )[:, kp, m, :, :]

            def w2_ap(q, m):
                return w2p_s[:].rearrange(
                    "p (q m i c) -> p q m i c", q=2, m=2, i=2
                )[:, q, m, :, :]

            # ---- passes of 2 context groups (+ events on pass 0) ----
            passes = [[0, 1]] + [[2 * i, 2 * i + 1] for i in range(1, GROUPS // 2)]
            for pi, grp in enumerate(passes):
                with_evt = pi == 0
                # MLP1
                s1t = {}
                for g in grp:
                    s1t[g] = s1pool.tile([128, 4 * 512], FP8, tag="s1", name=f"s1_{g}")
                if with_evt:
                    pEl = []
                pl = {}
                for m in range(4):
                    for g in grp:
                        pl[g] = pm1.tile([128, 512], F32, tag="pm1", name=f"p1_{g}_{m}")
                    if with_evt:
                        pE = pmisc.tile([128, BC], F32, tag="pmisc", name=f"pe_{m}")
                    for kp in range(NKP):
                        c, s = kp // 2, kp % 2
                        st, sp = kp == 0, kp == NKP - 1
                        for g in grp:
                            nc.tensor.matmul(
                                pl[g][:], w1_ap(kp, m), ctx_rhs(g, c, s, 512),
                                start=st, stop=sp, perf_mode=DR,
                            )
                        if with_evt:
                            nc.tensor.matmul(
                                pE[:], w1_ap(kp, m), evt_rhs(c, s),
                                start=st, stop=sp, perf_mode=DR,
                            )
                    for g in grp:
                        nc.scalar.activation(
                            s1t[g][:, 512 * m : 512 * (m + 1)], pl[g][:],
                            AF.Relu, bias=b1_s[:, m : m + 1], scale=1.0 / 16,
                        )
                    if with_evt:
                        nc.scalar.activation(
                            s1e_s[:, BC * m : BC * (m + 1)], pE[:],
                            AF.Relu, bias=b1_s[:, m : m + 1], scale=1.0 / 16,
                        )

                # MLP2
                s2t = {}
                for g in grp:
                    s2t[g] = s2pool.tile([128, 2 * 512], FP8, tag="s2", name=f"s2_{g}")
                for m in range(2):
                    p2 = {}
                    for g in grp:
                        p2[g] = pm2.tile([128, 512], F32, tag="pm2", name=f"p2_{g}_{m}")
                    if with_evt:
                        pE2 = pmisc.tile([128, BC], F32, tag="pmisc", name=f"pe2_{m}")
                    for q in range(2):
                        st, sp = q == 0, q == 1
                        for g in grp:
                            nc.tensor.matmul(
                                p2[g][:], w2_ap(q, m),
                                s1t[g][:].rearrange("p (mj x) -> p mj x", mj=4)[
                                    :, 2 * q : 2 * q + 2, :
                                ],
                                start=st, stop=sp, perf_mode=DR,
                            )
                        if with_evt:
                            nc.tensor.matmul(
                                pE2[:], w2_ap(q, m),
                                s1e_s[:].rearrange("p (mj x) -> p mj x", mj=4)[
                                    :, 2 * q : 2 * q + 2, :
                                ],
                                start=st, stop=sp, perf_mode=DR,
                            )
                    for g in grp:
                        nc.scalar.activation(
                            s2t[g][:, 512 * m : 512 * (m + 1)], p2[g][:],
                            AF.Relu, bias=b2_s[:, m : m + 1], scale=1.0 / 8,
                        )
                    if with_evt:
                        nc.scalar.activation(
                            sgram_s[:, BC * m : BC * (m + 1)], pE2[:],
                            AF.Relu, bias=b2_s[:, m : m + 1], scale=1.0 / 8,
                        )

                if with_evt:
                    # event extras: squares, |e|^2, predicates, variances
                    nc.vector.tensor_mul(cse_s[:], sgram_s[:], sgram_s[:])
                    pne = pmisc.tile([BC, 1], F32, tag="pmisc", name="pne")
                    nc.tensor.matmul(
                        pne[:],
                        cse_s[:].rearrange("p (m c) -> p m c", m=2),
                        ones2c_s[:, 0:2].rearrange("p (o i) -> p i o", i=2),
                        start=True, stop=True, perf_mode=DR,
                    )
                    nc.scalar.copy(ne2_s[:], pne[:])

                    nc.scalar.copy(
                        predb_s[:].rearrange("p (s i l) -> p s i l", s=2, i=2),
                        xe_s[:].bitcast(FP8).rearrange(
                            "p (s r i) -> p s i r", s=2, i=2
                        )[:, :, :, 128 : 128 + BC],
                    )
                    pvar = pmisc.tile([BC, 9], F32, tag="pmisc", name="pvar")
                    for f in range(4):
                        nc.tensor.matmul(
                            pvar[:],
                            predb_s[:].rearrange("p (f l) -> p f l", f=4)[:, f, :],
                            wvp_s[:].rearrange("p (f v) -> p f v", f=4)[:, f, :],
                            start=(f == 0), stop=(f == 3),
                        )
                    ez = smpool.tile([BC, 9], F32, tag="sm9", name="ez")
                    nc.scalar.activation(ez[:], pvar[:], AF.Exp, scale=1.0 / 8)
                    ezb = smpool.tile([BC, 9], F32, tag="sm9", name="ezb")
                    nc.vector.tensor_mul(ezb[:], ez[:], ebv_s[:])
                    ez1 = smpool.tile([BC, 9], F32, tag="sm9", name="ez1")
                    nc.vector.tensor_scalar_add(ez1[:], ezb[:], 1.0)
                    var = smpool.tile([BC, 9], F32, tag="sm9", name="var")
                    nc.scalar.activation(var[:], ez1[:], AF.Ln)
                    rv = smpool.tile([BC, 9], F32, tag="sm9", name="rv")
                    nc.vector.reciprocal(rv[:], var[:])
                    qd = smpool.tile([BC, 9], F32, tag="sm9", name="qd")
                    nc.vector.tensor_mul(qd[:], ndsq_s[:], rv[:])
                    nc.scalar.activation(F_s[:, 0:9], qd[:], AF.Exp)

                # dots and |c|^2 per batch: [p,2,1] stationaries, outputs on
                # partition 0, one DMA per group shifts to batch-row layout
                for g in grp:
                    csq = csqpool.tile(
                        [128, 2 * 512], FP8, tag="csq", name=f"csq_{g}"
                    )
                    nc.vector.tensor_mul(csq[:], s2t[g][:], s2t[g][:])
                    pdot = pgpool.tile([1, 512], F32, tag="pg", name=f"pd_{g}")
                    pnrm = pgpool.tile([1, 512], F32, tag="pg", name=f"pn_{g}")
                    sg_v = sgram_s[:].rearrange("p (m c) -> p m c", m=2)
                    s2_v = s2t[g][:].rearrange("p (m x) -> p m x", m=2)
                    cs_v = csq[:].rearrange("p (m x) -> p m x", m=2)
                    on_v = ones2c_s[:].rearrange("p (i x) -> p i x", i=2)[:, :, 0:1]
                    for s in range(4):
                        lane = 4 * g + s
                        nc.tensor.matmul(
                            pdot[:, 128 * s : 128 * (s + 1)],
                            sg_v[:, :, lane : lane + 1],
                            s2_v[:, :, 128 * s : 128 * (s + 1)],
                            start=True, stop=True, perf_mode=DR,
                        )
                        nc.tensor.matmul(
                            pnrm[:, 128 * s : 128 * (s + 1)],
                            on_v,
                            cs_v[:, :, 128 * s : 128 * (s + 1)],
                            start=True, stop=True, perf_mode=DR,
                        )
                    nc.scalar.copy(drow_s[:, 512 * g : 512 * (g + 1)], pdot[:])
                    nc.vector.tensor_copy(
                        out=nrow_s[:, 512 * g : 512 * (g + 1)], in_=pnrm[:]
                    )
                # prefetch the gathers this pass's successors will need
                for g in grp:
                    issue_gather(g + 4)

            # ---- tail: cosine, kernel pooling, final score ----
            nc.sync.dma_start(traw_s[:], drow_s[:])
            nc.sync.dma_start(ncsq0_s[:], nrow_s[:])
            prodn = smpool.tile([BC, 128], F32, tag="smT", name="prodn")
            nc.vector.tensor_tensor(
                out=prodn[:], in0=ncsq0_s[:],
                in1=ne2_s[:].broadcast_to([BC, 128]),
                op=mybir.AluOpType.mult,
            )
            prod1 = smpool.tile([BC, 128], F32, tag="smT", name="prod1")
            nc.vector.tensor_scalar_add(prod1[:], prodn[:], 1e-20)
            rec_s = smpool.tile([BC, 128], F32, tag="smT", name="rec")
            nc.vector.reciprocal(rec_s[:], prod1[:])
            nf_s = smpool.tile([BC, 128], F32, tag="smT", name="nf")
            nc.scalar.activation(nf_s[:], rec_s[:], AF.Sqrt)
            nc.vector.tensor_mul(trans_s[:], traw_s[:], nf_s[:])

            for k in range(NK):
                sq = smpool.tile([BC, 128], F32, tag="smT", name=f"sq{k}")
                nc.scalar.activation(
                    sq[:], trans_s[:], AF.Square, bias=mub_s[:, k : k + 1]
                )
                arg = smpool.tile([BC, 128], F32, tag="smT", name=f"arg{k}")
                nc.vector.tensor_scalar(
                    arg[:], sq[:], -1.0 / (2.0 * SIGMAS[k] ** 2), -87.0,
                    mybir.AluOpType.mult, mybir.AluOpType.max,
                )
                ek = smpool.tile([BC, 128], F32, tag="smT", name=f"ek{k}")
                nc.scalar.activation(ek[:], arg[:], AF.Exp)
                nc.vector.reduce_sum(
                    out=pooled_s[:, k : k + 1], in_=ek[:], axis=mybir.AxisListType.X
                )

            poolc = smpool.tile([BC, NK], F32, tag="smK", name="poolc")
            nc.vector.tensor_scalar_max(poolc[:], pooled_s[:], 1e-10)
            nc.scalar.activation(F_s[:, 9 + NF :], poolc[:], AF.Ln)

            fw = smpool.tile([BC, FD], F32, tag="smK", name="fw")
            nc.vector.tensor_mul(fw[:], F_s[:], wcr_s[:])
            sc = smpool.tile([BC, 1], F32, tag="smS", name="sc")
            nc.vector.reduce_sum(out=sc[:], in_=fw[:], axis=mybir.AxisListType.X)
            sig = smpool.tile([BC, 1], F32, tag="smS", name="sig")
            nc.scalar.activation(sig[:], sc[:], AF.Sigmoid, bias=bcr_s[:])
            nc.sync.dma_start(out_d.ap(), sig[:])

    nc.compile()

    # Spread SWDGE gathers across the 4 queues (ucode locks each DMASW
    # semaphore lane to one queue; lanes are assigned round-robin in
    # scheduled order).
    import re as _re
    for blk in nc.m.functions[0].blocks:
        for inst in blk.instructions:
            if type(inst).__name__ == "InstDMAGatherAnt":
                for u in inst.sync_info.on_update:
                    m = _re.match(r"DMASW(\d+)_", u.ant_name or "")
                    if m:
                        inst.queue_num = int(m.group(1)) % 4
                        break

    _dedup_ldweights(nc)

    _PROGRAM_CACHE[True] = nc
    return nc


def _ldw_sig(inst):
    a = inst.ins[0]
    return (
        a.memref,
        a.offset,
        tuple(tuple(d) for d in a.ap),
        getattr(inst, "perf_mode", None),
        getattr(inst, "tile_position", None),
        getattr(inst, "tile_size", None),
        getattr(inst, "is_transpose", None),
    )


def _dedup_ldweights(nc):
    """Remove InstLdweights that reload the stationary operand already in the
    PE array.  The compile pass splits every matmul into LDWEIGHTS+MATMUL;
    back-to-back matmuls sharing weights then pay a redundant ~180ns load.
    Conservative: only drops loads carrying no semaphore waits/updates, so
    cross-engine ordering is untouched."""
    dropped = 0
    for blk in nc.m.functions[0].blocks:
        cur = None          # signature currently in the array
        keep = []
        for inst in blk.instructions:
            nm = type(inst).__name__
            if nm == "InstLdweights":
                sig = _ldw_sig(inst)
                si = inst.sync_info
                if sig == cur and (
                    si is None or (not si.on_wait and not si.on_update)
                ):
                    dropped += 1
                    continue
                cur = sig
            keep.append(inst)
        blk.instructions = keep
    return dropped


def _wrap16(flat_idx):
    """int16 index list -> (128, n/16) tile layout replicated into 8 stripes."""
    n = flat_idx.shape[0]
    t = np.zeros((16, n // 16), np.int16)
    t[np.arange(n) % 16, np.arange(n) // 16] = flat_idx
    return np.tile(t, (8, 1))


FP8NP = ml_dtypes.float8_e4m3fn


def _prep_core_inputs(inputs, core, fast=True, table8=None):
    """Host-side shard + weight re-layouts for one core."""
    W1 = np.asarray(inputs["W1"], np.float32)
    W2 = np.asarray(inputs["W2"], np.float32)
    Wv = np.asarray(inputs["Wv"], np.float32)
    Wc = np.asarray(inputs["Wc"], np.float32)
    b1 = np.asarray(inputs["b1"], np.float32)
    b2 = np.asarray(inputs["b2"], np.float32)
    bv = np.asarray(inputs["bv"], np.float32)
    bc = np.asarray(inputs["bc"], np.float32)

    sl = slice(core * BC, (core + 1) * BC)
    ev = np.asarray(inputs["batch_event"][sl], np.int64)          # (BC, C)
    feats = np.asarray(inputs["batch_features"][sl], np.float32)  # (BC, NF)
    dists = np.asarray(inputs["batch_distances"][sl], np.float32) # (BC, 9)
    ctx = np.asarray(inputs["batch_context"][sl], np.int64)       # (BC, N, C)

    if table8 is None:
        table8 = (np.asarray(inputs["event_table"], np.float32) * 8.0).astype(FP8NP)

    # compact fp8 table, rows padded to 512 bytes, viewed as int16 units
    allidx = np.concatenate([ctx.reshape(-1), ev.reshape(-1)])
    uniq, inv = np.unique(allidx, return_inverse=True)
    assert len(uniq) <= CT
    ctab8 = np.zeros((CT, 2 * RU), FP8NP)
    ctab8[: len(uniq), :E] = table8[uniq]
    rctx = inv[: ctx.size].astype(np.int16).reshape(BC, N, C)
    rev = inv[ctx.size :].astype(np.int16).reshape(BC, C)

    # context gathers: per (group g, comp c), 512 idxs ordered (s, n)
    # -> row for (b=4g+s, n)
    ci = rctx.reshape(GROUPS, 4, N, C).transpose(0, 3, 1, 2)  # g, c, s, n
    cidx = np.concatenate(
        [
            _wrap16(ci[g, c].reshape(-1))
            for g in range(GROUPS)
            for c in range(C)
        ],
        axis=1,
    )
    # event gather: idx j = c*128 + lane; lanes >= BC gather row 0
    ei = np.zeros((C, 128), np.int16)
    ei[:, :BC] = rev.T

    # W1 packed for DoubleRow: [p, kp(c,s), m, i, mcol]
    W1x = (8.0 * W1).astype(np.float32)          # (H1, C*E)
    W2x = (8.0 * W2).astype(np.float32)          # (H2, H1)
    p_i = np.arange(128)
    w1p = np.zeros((128, NKP, 4, 2, 128), np.float32)
    for c in range(C):
        for s in range(2):
            eloc = 256 * s + 2 * p_i[:, None] + np.arange(2)[None, :]  # (128, 2)
            valid = eloc < E
            src = W1x[:, c * E + np.minimum(eloc, E - 1)]  # (H1, 128, 2)
            src = src * valid[None, :, :]
            # (m, mcol, p, i) -> [p, m, i, mcol]
            blk = src.reshape(4, 128, 128, 2).transpose(2, 0, 3, 1)
            w1p[:, 2 * c + s] = blk
    w2p = np.zeros((128, 2, 2, 2, 128), np.float32)
    for q in range(2):
        for i in range(2):
            src = W2x[:, 128 * (2 * q + i) + p_i]      # (H2, 128)
            w2p[:, q, :, i, :] = src.reshape(2, 128, 128).transpose(2, 0, 1)
    wvp = np.zeros((128, 4, 9), np.float32)
    for f in range(4):
        eloc = 256 * (f // 2) + 2 * p_i + (f % 2)
        valid = eloc < E
        wvp[:, f, :] = Wv[:, np.minimum(eloc, E - 1)].T * valid[:, None]

    wc_r = np.concatenate(
        [Wc[0, 0:9], Wc[0, 9 : 9 + NF], Wc[0, 9 + NF :] * 0.01]
    ).astype(np.float32)

    m = {
        "ctab": np.ascontiguousarray(ctab8).view(np.int16),
        "cidx": np.ascontiguousarray(cidx),
        "eidx": np.ascontiguousarray(_wrap16(ei.reshape(-1))),
        "w1p": w1p.reshape(128, -1).astype(FP8NP),
        "w2p": w2p.reshape(128, -1).astype(FP8NP),
        "wvp": wvp.reshape(128, -1).astype(ml_dtypes.bfloat16),
        "b1d": np.ascontiguousarray(4.0 * b1.reshape(4, 128).T),
        "b2d": np.ascontiguousarray(4.0 * b2.reshape(2, 128).T),
        "ebv": np.tile(np.exp(bv)[None, :], (BC, 1)).astype(np.float32),
        "ndsq": np.ascontiguousarray(-(dists * dists)),
        "featd": np.ascontiguousarray(feats),
        "wcr": np.tile(wc_r[None, :], (BC, 1)),
        "bcr": np.full((BC, 1), bc[0], np.float32),
    }
    return m


def _numpy_reference(inputs):
    """Pure-host fallback (unreachable for the spec's random fill)."""
    t = np.asarray(inputs["event_table"], np.float32)
    W1 = np.asarray(inputs["W1"], np.float32); b1 = np.asarray(inputs["b1"], np.float32)
    W2 = np.asarray(inputs["W2"], np.float32); b2 = np.asarray(inputs["b2"], np.float32)
    Wv = np.asarray(inputs["Wv"], np.float32); bv = np.asarray(inputs["bv"], np.float32)
    Wc = np.asarray(inputs["Wc"], np.float32); bc = np.asarray(inputs["bc"], np.float32)
    be = np.asarray(inputs["batch_event"], np.int64)
    bf = np.asarray(inputs["batch_features"], np.float32)
    bd = np.asarray(inputs["batch_distances"], np.float32)
    bx = np.asarray(inputs["batch_context"], np.int64)
    ee = t[be]                                    # (B, C, E)
    ce = t[bx]                                    # (B, N, C, E)
    pred = ee[:, 1, :]
    zv = pred @ Wv.T + bv
    var = np.log1p(np.exp(zv))
    de = np.exp(-(bd * bd) / var)
    ex = np.concatenate([de, bf], axis=1)

    def mlp(x):
        h = np.maximum(x @ W1.T + b1, 0.0)
        return np.maximum(h @ W2.T + b2, 0.0)

    er = mlp(ee.reshape(B, C * E))                # (B, H2)
    cr = mlp(ce.reshape(B * N, C * E)).reshape(B, N, H2)
    ern = er / np.maximum(np.linalg.norm(er, axis=-1, keepdims=True), 1e-12)
    crn = cr / np.maximum(np.linalg.norm(cr, axis=-1, keepdims=True), 1e-12)
    tr = np.einsum("bd,bnd->bn", ern, crn)        # (B, N)
    mus = np.array(MUS, np.float32)
    sig = np.array(SIGMAS, np.float32)
    kk = np.exp(-((tr[..., None] - mus) ** 2) / (2.0 * sig ** 2))
    pooled = kk.sum(axis=1)
    kp = np.log(np.clip(pooled, 1e-10, None)) * 0.01
    af = np.concatenate([ex, kp], axis=1)
    sc = af @ Wc[0] + bc[0]
    return (1.0 / (1.0 + np.exp(-sc)))[:, None].astype(np.float32)


def kernel(**inputs) -> np.ndarray:
    ctx = np.asarray(inputs["batch_context"], np.int64)
    ev = np.asarray(inputs["batch_event"], np.int64)
    for core in range(NCORES):
        sl = slice(core * BC, (core + 1) * BC)
        nuniq = len(np.unique(np.concatenate(
            [ctx[sl].reshape(-1), ev[sl].reshape(-1)])))
        if nuniq > CT:
            return _numpy_reference(inputs)
    nc = _build_program(True)
    table8 = (np.asarray(inputs["event_table"], np.float32) * 8.0).astype(FP8NP)
    in_maps = [
        _prep_core_inputs(inputs, core, True, table8) for core in range(NCORES)
    ]
    res = run_bass_kernel_spmd(nc, in_maps, core_ids=list(range(NCORES)))
    return np.concatenate([r["out"] for r in res.results], axis=0)


if __name__ == "__main__":
    nc = _build_program(True)
    print("program built ok")


# revision 23
# speedup vs baseline: 1.4175x; 1.1273x over previous
"""Trainium2 Bass kernel for nn_EventPairCompositionModel (fp8 DoubleRow).

Strategy (data-parallel over batch, 8 cores, B=512 -> 64 per core):
  - Host builds a per-core compact table of COMPONENT-PAIR rows: the ~16K
    unique (idx[2c], idx[2c+1]) pairs each shard touches, stored as fp8e4m3
    (x8 scale) 600-elem rows padded to 768 bytes and DECLARED int16 so the
    SWDGE transpose-gather's 16-bit granularity lands fp8 element pairs
    (2u, 2u+1) on partition u%128 -- exactly the [K, 2, N] layout
    MatmulPerfMode.DoubleRow wants.  Pair rows mean 6 DoubleRow K-pairs per
    m-tile (vs 8 for single rows) and half the gather indices (the SWDGE
    ucode is per-index bound).
  - Context MLP (1200->512->256) entirely in fp8 DoubleRow (2 K-rows/cycle),
    processing 4 groups of 512 (b,n) pairs per weight pass so each stationary
    load amortizes over 4 matmuls; redundant LDWEIGHTS are removed by a
    post-compile pass.
  - The 64 event tuples ride pass 0 as a narrow (64-col) extra group.
  - Cosine numerators/|c|^2 via per-batch fp8 DR matmuls ([p,2,1]
    stationaries) landing on partition 0, staged to [1, 8192] rows and
    reshaped to [64 batch, 128 ctx] by one DMA; KNRM pooling on [64,128].
  - Distance-kernel path in bf16 off the event gather; final linear+sigmoid
    in [64 batch partitions, feature] layout.
All 8 cores run the identical program on their own batch shard (SPMD).
Host falls back to a numpy reference if a shard exceeds the int16 compact
table (cannot happen for the random fill).
"""

import numpy as np
import ml_dtypes

import concourse.bacc as bacc
import concourse.bass as bass
import concourse.tile as tile
import concourse.mybir as mybir
from concourse.bass_utils import run_bass_kernel_spmd
from concourse import library_config

F32 = mybir.dt.float32
BF16 = mybir.dt.bfloat16
FP8 = mybir.dt.float8e4
I16 = mybir.dt.int16
AF = mybir.ActivationFunctionType
DR = mybir.MatmulPerfMode.DoubleRow

# Problem shapes (hardcoded per spec)
B, N, C, E = 512, 128, 4, 300
V = 50000
H1, H2 = 512, 256
NF, NK = 8, 11
NCORES = 8
BC = B // NCORES          # 64 batches per core
CT = 32768                # compact pair-table rows (int16-indexable)
EP = 600                  # elems per pair row (2 components)
RU = 384                  # int16 units per table row (768 fp8 elems)
GROUPS = (BC * N) // 512  # 16 groups of 512 (b,n) pairs
NKP = 6                   # MLP1 K-pairs per m-tile (2 pair-comps x 3 slots)
FD = 28                   # features: 9 dist + 8 extracted + 11 kp
PASSES = 4                # groups per weight pass
NPASS = GROUPS // PASSES

MUS = [1.0, 0.9, 0.7, 0.5, 0.3, 0.1, -0.1, -0.3, -0.5, -0.7, -0.9]
SIGMAS = [1e-3] + [0.1] * 10

_PROGRAM_CACHE = {}


def _build_program(fast: bool = True):
    if True in _PROGRAM_CACHE:
        return _PROGRAM_CACHE[True]

    nc = bacc.Bacc("TRN2", target_bir_lowering=False, debug=False, num_swdge_queues=4)

    # ---- DRAM I/O ----
    ctab = nc.dram_tensor("ctab", (CT, RU), I16, kind="ExternalInput")
    cidx = nc.dram_tensor("cidx", (128, GROUPS * 2 * 32), I16, kind="ExternalInput")
    eidx = nc.dram_tensor("eidx", (128, 16), I16, kind="ExternalInput")
    w1p = nc.dram_tensor("w1p", (128, NKP * 4 * 256), FP8, kind="ExternalInput")
    w2p = nc.dram_tensor("w2p", (128, 2 * 2 * 256), FP8, kind="ExternalInput")
    wvp = nc.dram_tensor("wvp", (128, 4 * 9), BF16, kind="ExternalInput")
    b1d = nc.dram_tensor("b1d", (128, 4), F32, kind="ExternalInput")
    b2d = nc.dram_tensor("b2d", (128, 2), F32, kind="ExternalInput")
    ebv = nc.dram_tensor("ebv", (BC, 9), F32, kind="ExternalInput")
    ndsq = nc.dram_tensor("ndsq", (BC, 9), F32, kind="ExternalInput")
    featd = nc.dram_tensor("featd", (BC, NF), F32, kind="ExternalInput")
    wcr = nc.dram_tensor("wcr", (BC, FD), F32, kind="ExternalInput")
    bcr = nc.dram_tensor("bcr", (BC, 1), F32, kind="ExternalInput")
    out_d = nc.dram_tensor("out", (BC, 1), F32, kind="ExternalOutput")

    with tile.TileContext(nc) as tc:
        with (
            tc.tile_pool(name="consts", bufs=1) as cpool,
            tc.tile_pool(name="xt", bufs=2 * PASSES) as xtpool,
            tc.tile_pool(name="s1", bufs=PASSES + 2) as s1pool,
            tc.tile_pool(name="s2", bufs=PASSES + 2) as s2pool,
            tc.tile_pool(name="csq", bufs=PASSES + 2) as csqpool,
            tc.tile_pool(name="small", bufs=2) as smpool,
            tc.tile_pool(name="pm1", bufs=PASSES, space="PSUM") as pm1,
            tc.tile_pool(name="pm2", bufs=2, space="PSUM") as pm2,
            tc.tile_pool(name="pg", bufs=2, space="PSUM") as pgpool,
        ):
            nc.gpsimd.load_library(library_config.mlp)

            # ---- load constants ----
            cidx_s = cpool.tile([128, GROUPS * 2 * 32], I16)
            nc.sync.dma_start(cidx_s[:], cidx.ap())
            eidx_s = cpool.tile([128, 16], I16)
            nc.sync.dma_start(eidx_s[:], eidx.ap())
            w1p_s = cpool.tile([128, NKP * 4 * 256], FP8)
            nc.sync.dma_start(w1p_s[:], w1p.ap())
            w2p_s = cpool.tile([128, 2 * 2 * 256], FP8)
            nc.scalar.dma_start(w2p_s[:], w2p.ap())
            wvp_s = cpool.tile([128, 4 * 9], BF16)
            nc.scalar.dma_start(wvp_s[:], wvp.ap())
            b1_s = cpool.tile([128, 4], F32)
            nc.sync.dma_start(b1_s[:], b1d.ap())
            b2_s = cpool.tile([128, 2], F32)
            nc.sync.dma_start(b2_s[:], b2d.ap())
            ebv_s = cpool.tile([BC, 9], F32)
            nc.scalar.dma_start(ebv_s[:], ebv.ap())
            ndsq_s = cpool.tile([BC, 9], F32)
            nc.scalar.dma_start(ndsq_s[:], ndsq.ap())
            wcr_s = cpool.tile([BC, FD], F32)
            nc.scalar.dma_start(wcr_s[:], wcr.ap())
            bcr_s = cpool.tile([BC, 1], F32)
            nc.scalar.dma_start(bcr_s[:], bcr.ap())
            F_s = cpool.tile([BC, FD], F32)
            nc.sync.dma_start(F_s[:, 9 : 9 + NF], featd.ap())

            # ones pair for |c|^2 matmuls; K-group dim at stride 16 to satisfy
            # the dual-fp8 LDWEIGHTS AP restriction (pair step % 16 == 0)
            ones2c_s = cpool.tile([128, 32], FP8)
            nc.vector.memset(ones2c_s[:], 1.0)
            mub_s = cpool.tile([BC, NK], F32)
            for k in range(NK):
                nc.vector.memset(mub_s[:, k : k + 1], -MUS[k])

            # persistent accumulators
            traw_s = cpool.tile([BC, 128], F32)        # 16*dot per (b, n)
            drow_s = cpool.tile([1, 512 * GROUPS], F32)  # dots, (g,s,n) on part 0
            nrow_s = cpool.tile([1, 512 * GROUPS], F32)  # |c|^2 likewise
            sgram_s = cpool.tile([128, 2 * BC], FP8)   # event reprs [p, m(2), 64]
            cse_s = cpool.tile([128, 2 * BC], FP8)     # their squares
            s1e_s = cpool.tile([128, 4 * BC], FP8)     # event s1 [p, mj(4), 64]
            predb_s = cpool.tile([128, 4 * BC], BF16)  # predicates [p, f(4), 64]
            ne2_s = cpool.tile([BC, 1], F32)
            ncsq0_s = cpool.tile([BC, 128], F32)
            trans_s = cpool.tile([BC, 128], F32)
            pooled_s = cpool.tile([BC, NK], F32)

            # ---- gathers (issued lazily so pool-slot reuse stays WAR-safe) ----
            xe_s = cpool.tile([128, 3 * 256], I16)   # event pair gather
            nc.gpsimd.dma_gather(
                out_ap=xe_s[:].rearrange("p (s i) -> p s i", s=3),
                in_ap=ctab.ap(),
                idxs_ap=eidx_s[:],
                num_idxs=256,
                num_idxs_reg=256,
                elem_size=RU,
                transpose=True,
            )
            xts = {}

            def issue_gather(g):
                # one 512-idx gather per component pair (512-idx bursts fit
                # the SWDGE descriptor-ring carveout; 2048 would deadlock it)
                if g >= GROUPS:
                    return
                xt = xtpool.tile([128, 2 * 3 * 512], I16, tag="xt", name=f"xt{g}")
                xv = xt[:].rearrange("p (cp s r) -> p cp s r", cp=2, s=3)
                for cp in range(2):
                    nc.gpsimd.dma_gather(
                        out_ap=xv[:, cp, :, :],
                        in_ap=ctab.ap(),
                        idxs_ap=cidx_s[:, 32 * (2 * g + cp) : 32 * (2 * g + cp + 1)],
                        num_idxs=512,
                        num_idxs_reg=512,
                        elem_size=RU,
                        transpose=True,
                    )
                xts[g] = xt

            for g in range(2 * PASSES):
                issue_gather(g)

            def ctx_rhs(g, cp, uj, w):
                # [p, 2(byte), w cols] fp8 view: group g, comp-pair cp, slot uj
                v = xts[g][:].bitcast(FP8).rearrange(
                    "p (cp s r i) -> p cp s i r", cp=2, s=3, i=2
                )
                return v[:, cp, uj, :, 0:w]

            def evt_rhs(cp, uj):
                v = xe_s[:].bitcast(FP8).rearrange("p (s r i) -> p s i r", s=3, i=2)
                return v[:, uj, :, 128 * cp : 128 * cp + BC]

            def w1_ap(kp, m):
                return w1p_s[:].rearrange(
                    "p (kp m i c) -> p kp m i c", kp=NKP, m=4, i=2
                )[:, kp, m, :, :]

            def w2_ap(q, m):
                return w2p_s[:].rearrange(
                    "p (q m i c) -> p q m i c", q=2, m=2, i=2
                )[:, q, m, :, :]

            # ---- passes of PASSES context groups (+ events on pass 0) ----
            for pi in range(NPASS):
                grp = list(range(PASSES * pi, PASSES * (pi + 1)))
                with_evt = pi == 0
                # MLP1
                s1t = {}
                for g in grp:
                    s1t[g] = s1pool.tile([128, 4 * 512], FP8, tag="s1", name=f"s1_{g}")
                pl = {}
                for m in range(4):
                    for g in grp:
                        pl[g] = pm1.tile([128, 512], F32, tag="pm1", name=f"p1_{g}_{m}")
                    if with_evt:
                        pE = pm2.tile([128, BC], F32, tag="pm2", name=f"pe_{m}")
                    for kp in range(NKP):
                        cp, uj = kp // 3, kp % 3
                        st, sp = kp == 0, kp == NKP - 1
                        for g in grp:
                            nc.tensor.matmul(
                                pl[g][:], w1_ap(kp, m), ctx_rhs(g, cp, uj, 512),
                                start=st, stop=sp, perf_mode=DR,
                            )
                        if with_evt:
                            nc.tensor.matmul(
                                pE[:], w1_ap(kp, m), evt_rhs(cp, uj),
                                start=st, stop=sp, perf_mode=DR,
                            )
                    for g in grp:
                        nc.scalar.activation(
                            s1t[g][:, 512 * m : 512 * (m + 1)], pl[g][:],
                            AF.Relu, bias=b1_s[:, m : m + 1], scale=1.0 / 16,
                        )
                    if with_evt:
                        nc.scalar.activation(
                            s1e_s[:, BC * m : BC * (m + 1)], pE[:],
                            AF.Relu, bias=b1_s[:, m : m + 1], scale=1.0 / 16,
                        )

                # MLP2 in half-passes of 2 groups (PSUM budget)
                s2t = {}
                for g in grp:
                    s2t[g] = s2pool.tile([128, 2 * 512], FP8, tag="s2", name=f"s2_{g}")
                for half in range(PASSES // 2):
                    gh = grp[2 * half : 2 * half + 2]
                    for m in range(2):
                        p2 = {}
                        for g in gh:
                            p2[g] = pm2.tile(
                                [128, 512], F32, tag="pm2", name=f"p2_{g}_{m}"
                            )
                        if with_evt and half == 0:
                            pE2 = pm2.tile([128, BC], F32, tag="pm2", name=f"pe2_{m}")
                        for q in range(2):
                            st, sp = q == 0, q == 1
                            for g in gh:
                                nc.tensor.matmul(
                                    p2[g][:], w2_ap(q, m),
                                    s1t[g][:].rearrange("p (mj x) -> p mj x", mj=4)[
                                        :, 2 * q : 2 * q + 2, :
                                    ],
                                    start=st, stop=sp, perf_mode=DR,
                                )
                            if with_evt and half == 0:
                                nc.tensor.matmul(
                                    pE2[:], w2_ap(q, m),
                                    s1e_s[:].rearrange("p (mj x) -> p mj x", mj=4)[
                                        :, 2 * q : 2 * q + 2, :
                                    ],
                                    start=st, stop=sp, perf_mode=DR,
                                )
                        for g in gh:
                            nc.scalar.activation(
                                s2t[g][:, 512 * m : 512 * (m + 1)], p2[g][:],
                                AF.Relu, bias=b2_s[:, m : m + 1], scale=1.0 / 8,
                            )
                        if with_evt and half == 0:
                            nc.scalar.activation(
                                sgram_s[:, BC * m : BC * (m + 1)], pE2[:],
                                AF.Relu, bias=b2_s[:, m : m + 1], scale=1.0 / 8,
                            )

                    if with_evt and half == 0:
                        # event extras: squares, |e|^2, predicates, variances
                        nc.vector.tensor_mul(cse_s[:], sgram_s[:], sgram_s[:])
                        pne = pgpool.tile([BC, 1], F32, tag="pg", name="pne")
                        nc.tensor.matmul(
                            pne[:],
                            cse_s[:].rearrange("p (m c) -> p m c", m=2),
                            ones2c_s[:, 0:2].rearrange("p (o i) -> p i o", i=2),
                            start=True, stop=True, perf_mode=DR,
                        )
                        nc.scalar.copy(ne2_s[:], pne[:])

                        # predicates: elems 300..599 of the cp=0 pair rows
                        nc.scalar.copy(
                            predb_s[:].rearrange("p (s i l) -> p s i l", s=2, i=2),
                            xe_s[:].bitcast(FP8).rearrange(
                                "p (s r i) -> p s i r", s=3, i=2
                            )[:, 1:3, :, 0:BC],
                        )
                        pvar = pgpool.tile([BC, 9], F32, tag="pg", name="pvar")
                        for f in range(4):
                            nc.tensor.matmul(
                                pvar[:],
                                predb_s[:].rearrange("p (f l) -> p f l", f=4)[:, f, :],
                                wvp_s[:].rearrange("p (f v) -> p f v", f=4)[:, f, :],
                                start=(f == 0), stop=(f == 3),
                            )
                        ez = smpool.tile([BC, 9], F32, tag="sm9", name="ez")
                        nc.scalar.activation(ez[:], pvar[:], AF.Exp, scale=1.0 / 8)
                        ezb = smpool.tile([BC, 9], F32, tag="sm9", name="ezb")
                        nc.vector.tensor_mul(ezb[:], ez[:], ebv_s[:])
                        ez1 = smpool.tile([BC, 9], F32, tag="sm9", name="ez1")
                        nc.vector.tensor_scalar_add(ez1[:], ezb[:], 1.0)
                        var = smpool.tile([BC, 9], F32, tag="sm9", name="var")
                        nc.scalar.activation(var[:], ez1[:], AF.Ln)
                        rv = smpool.tile([BC, 9], F32, tag="sm9", name="rv")
                        nc.vector.reciprocal(rv[:], var[:])
                        qd = smpool.tile([BC, 9], F32, tag="sm9", name="qd")
                        nc.vector.tensor_mul(qd[:], ndsq_s[:], rv[:])
                        nc.scalar.activation(F_s[:, 0:9], qd[:], AF.Exp)

                    # dots and |c|^2 per batch for this half's groups
                    for g in gh:
                        csq = csqpool.tile(
                            [128, 2 * 512], FP8, tag="csq", name=f"csq_{g}"
                        )
                        nc.vector.tensor_mul(csq[:], s2t[g][:], s2t[g][:])
                        pdot = pgpool.tile([1, 512], F32, tag="pg", name=f"pd_{g}")
                        pnrm = pgpool.tile([1, 512], F32, tag="pg", name=f"pn_{g}")
                        sg_v = sgram_s[:].rearrange("p (m c) -> p m c", m=2)
                        s2_v = s2t[g][:].rearrange("p (m x) -> p m x", m=2)
                        cs_v = csq[:].rearrange("p (m x) -> p m x", m=2)
                        on_v = ones2c_s[:].rearrange("p (i x) -> p i x", i=2)[
                            :, :, 0:1
                        ]
                        for s in range(4):
                            lane = 4 * g + s
                            nc.tensor.matmul(
                                pdot[:, 128 * s : 128 * (s + 1)],
                                sg_v[:, :, lane : lane + 1],
                                s2_v[:, :, 128 * s : 128 * (s + 1)],
                                start=True, stop=True, perf_mode=DR,
                            )
                            nc.tensor.matmul(
                                pnrm[:, 128 * s : 128 * (s + 1)],
                                on_v,
                                cs_v[:, :, 128 * s : 128 * (s + 1)],
                                start=True, stop=True, perf_mode=DR,
                            )
                        nc.vector.tensor_copy(
                            out=drow_s[:, 512 * g : 512 * (g + 1)], in_=pdot[:]
                        )
                        nc.vector.tensor_copy(
                            out=nrow_s[:, 512 * g : 512 * (g + 1)], in_=pnrm[:]
                        )
                # prefetch the gathers needed two passes ahead
                for g in grp:
                    issue_gather(g + 2 * PASSES)

            # ---- tail: cosine, kernel pooling, final score ----
            nc.sync.dma_start(traw_s[:], drow_s[:])
            nc.sync.dma_start(ncsq0_s[:], nrow_s[:])
            prodn = smpool.tile([BC, 128], F32, tag="smT", name="prodn")
            nc.vector.tensor_tensor(
                out=prodn[:], in0=ncsq0_s[:],
                in1=ne2_s[:].broadcast_to([BC, 128]),
                op=mybir.AluOpType.mult,
            )
            prod1 = smpool.tile([BC, 128], F32, tag="smT", name="prod1")
            nc.vector.tensor_scalar_add(prod1[:], prodn[:], 1e-20)
            rec_s = smpool.tile([BC, 128], F32, tag="smT", name="rec")
            nc.vector.reciprocal(rec_s[:], prod1[:])
            nf_s = smpool.tile([BC, 128], F32, tag="smT", name="nf")
            nc.scalar.activation(nf_s[:], rec_s[:], AF.Sqrt)
            nc.vector.tensor_mul(trans_s[:], traw_s[:], nf_s[:])

            for k in range(NK):
                sq = smpool.tile([BC, 128], F32, tag="smT", name=f"sq{k}")
                nc.scalar.activation(
                    sq[:], trans_s[:], AF.Square, bias=mub_s[:, k : k + 1]
                )
                arg = smpool.tile([BC, 128], F32, tag="smT", name=f"arg{k}")
                nc.vector.tensor_scalar(
                    arg[:], sq[:], -1.0 / (2.0 * SIGMAS[k] ** 2), -87.0,
                    mybir.AluOpType.mult, mybir.AluOpType.max,
                )
                ek = smpool.tile([BC, 128], F32, tag="smT", name=f"ek{k}")
                nc.scalar.activation(ek[:], arg[:], AF.Exp)
                nc.vector.reduce_sum(
                    out=pooled_s[:, k : k + 1], in_=ek[:], axis=mybir.AxisListType.X
                )

            poolc = smpool.tile([BC, NK], F32, tag="smK", name="poolc")
            nc.vector.tensor_scalar_max(poolc[:], pooled_s[:], 1e-10)
            nc.scalar.activation(F_s[:, 9 + NF :], poolc[:], AF.Ln)

            fw = smpool.tile([BC, FD], F32, tag="smK", name="fw")
            nc.vector.tensor_mul(fw[:], F_s[:], wcr_s[:])
            sc = smpool.tile([BC, 1], F32, tag="smS", name="sc")
            nc.vector.reduce_sum(out=sc[:], in_=fw[:], axis=mybir.AxisListType.X)
            sig = smpool.tile([BC, 1], F32, tag="smS", name="sig")
            nc.scalar.activation(sig[:], sc[:], AF.Sigmoid, bias=bcr_s[:])
            nc.sync.dma_start(out_d.ap(), sig[:])

    nc.compile()

    # Spread SWDGE gathers across the 4 queues (ucode locks each DMASW
    # semaphore lane to one queue; lanes are assigned round-robin in
    # scheduled order).
    import re as _re
    for blk in nc.m.functions[0].blocks:
        for inst in blk.instructions:
            if type(inst).__name__ == "InstDMAGatherAnt":
                for u in inst.sync_info.on_update:
                    m = _re.match(r"DMASW(\d+)_", u.ant_name or "")
                    if m:
                        inst.queue_num = int(m.group(1)) % 4
                        break

    _dedup_ldweights(nc)

    _PROGRAM_CACHE[True] = nc
    return nc


def _ldw_sig(inst):
    a = inst.ins[0]
    return (
        a.memref,
        a.offset,
        tuple(tuple(d) for d in a.ap),
        getattr(inst, "perf_mode", None),
        getattr(inst, "tile_position", None),
        getattr(inst, "tile_size", None),
        getattr(inst, "is_transpose", None),
    )


def _dedup_ldweights(nc):
    """Remove InstLdweights that reload the stationary operand already in the
    PE array.  The compile pass splits every matmul into LDWEIGHTS+MATMUL;
    back-to-back matmuls sharing weights then pay a redundant ~200ns load.
    Conservative: only drops loads carrying no semaphore waits/updates, so
    cross-engine ordering is untouched."""
    dropped = 0
    for blk in nc.m.functions[0].blocks:
        cur = None          # signature currently in the array
        keep = []
        for inst in blk.instructions:
            nm = type(inst).__name__
            if nm == "InstLdweights":
                sig = _ldw_sig(inst)
                si = inst.sync_info
                if sig == cur and (
                    si is None or (not si.on_wait and not si.on_update)
                ):
                    dropped += 1
                    continue
                cur = sig
            keep.append(inst)
        blk.instructions = keep
    return dropped


def _wrap16(flat_idx):
    """int16 index list -> (128, n/16) tile layout replicated into 8 stripes."""
    n = flat_idx.shape[0]
    t = np.zeros((16, n // 16), np.int16)
    t[np.arange(n) % 16, np.arange(n) // 16] = flat_idx
    return np.tile(t, (8, 1))


FP8NP = ml_dtypes.float8_e4m3fn


def _prep_core_inputs(inputs, core, fast=True, table8=None):
    """Host-side shard + weight re-layouts for one core."""
    W1 = np.asarray(inputs["W1"], np.float32)
    W2 = np.asarray(inputs["W2"], np.float32)
    Wv = np.asarray(inputs["Wv"], np.float32)
    Wc = np.asarray(inputs["Wc"], np.float32)
    b1 = np.asarray(inputs["b1"], np.float32)
    b2 = np.asarray(inputs["b2"], np.float32)
    bv = np.asarray(inputs["bv"], np.float32)
    bc = np.asarray(inputs["bc"], np.float32)

    sl = slice(core * BC, (core + 1) * BC)
    ev = np.asarray(inputs["batch_event"][sl], np.int64)          # (BC, C)
    feats = np.asarray(inputs["batch_features"][sl], np.float32)  # (BC, NF)
    dists = np.asarray(inputs["batch_distances"][sl], np.float32) # (BC, 9)
    ctx = np.asarray(inputs["batch_context"][sl], np.int64)       # (BC, N, C)

    if table8 is None:
        table8 = (np.asarray(inputs["event_table"], np.float32) * 8.0).astype(FP8NP)

    # component-pair keys: (idx0, idx1) and (idx2, idx3) per (b, n) / event
    ctxp = ctx.reshape(BC, N, 2, 2)          # (b, n, cp, which)
    evp = ev.reshape(BC, 2, 2)
    keys = np.concatenate(
        [
            (ctxp[..., 0] * np.int64(V + 1) + ctxp[..., 1]).reshape(-1),
            (evp[..., 0] * np.int64(V + 1) + evp[..., 1]).reshape(-1),
        ]
    )
    uniq, inv = np.unique(keys, return_inverse=True)
    assert len(uniq) <= CT
    u0 = (uniq // (V + 1)).astype(np.int64)
    u1 = (uniq % (V + 1)).astype(np.int64)
    ctab8 = np.zeros((CT, 2 * RU), FP8NP)
    ctab8[: len(uniq), 0:E] = table8[u0]
    ctab8[: len(uniq), E : 2 * E] = table8[u1]
    nctx = BC * N * 2
    rctx = inv[:nctx].astype(np.int16).reshape(BC, N, 2)
    rev = inv[nctx:].astype(np.int16).reshape(BC, 2)

    # context gathers: per (group g, comp-pair cp), 512 idxs ordered (s, n)
    ci = rctx.reshape(GROUPS, 4, N, 2).transpose(0, 3, 1, 2)  # g, cp, s, n
    cidx = np.concatenate(
        [
            _wrap16(ci[g, cp].reshape(-1))
            for g in range(GROUPS)
            for cp in range(2)
        ],
        axis=1,
    )
    # event gather: 256 idxs, j = cp*128 + lane; lanes >= BC gather row 0
    ei = np.zeros((2, 128), np.int16)
    ei[:, :BC] = rev.T

    # W1 packed for DoubleRow: [p, kp(cp,uj), m, i, mcol]
    W1x = (8.0 * W1).astype(np.float32)          # (H1, C*E)
    W2x = (8.0 * W2).astype(np.float32)          # (H2, H1)
    p_i = np.arange(128)
    w1p = np.zeros((128, NKP, 4, 2, 128), np.float32)
    for cp in range(2):
        for uj in range(3):
            # pair-row element index e in [0, 768); maps to W1 column
            e = 256 * uj + 2 * p_i[:, None] + np.arange(2)[None, :]  # (128, 2)
            comp = 2 * cp + (e >= E)
            off = e - E * (e >= E)
            valid = e < EP
            col = np.minimum(comp * E + off, C * E - 1)
            src = W1x[:, col] * valid[None, :, :]   # (H1, 128, 2)
            blk = src.reshape(4, 128, 128, 2).transpose(2, 0, 3, 1)
            w1p[:, 3 * cp + uj] = blk
    w2p = np.zeros((128, 2, 2, 2, 128), np.float32)
    for q in range(2):
        for i in range(2):
            src = W2x[:, 128 * (2 * q + i) + p_i]      # (H2, 128)
            w2p[:, q, :, i, :] = src.reshape(2, 128, 128).transpose(2, 0, 1)
    # predicates live at pair-row elems 300..599 (comp 1 of the cp=0 row):
    # f slots are (uj, i) for uj in {1, 2}
    wvp = np.zeros((128, 4, 9), np.float32)
    for f in range(4):
        e = 256 * (1 + f // 2) + 2 * p_i + (f % 2)
        k = e - E
        valid = (k >= 0) & (k < E)
        wvp[:, f, :] = Wv[:, np.clip(k, 0, E - 1)].T * valid[:, None]

    wc_r = np.concatenate(
        [Wc[0, 0:9], Wc[0, 9 : 9 + NF], Wc[0, 9 + NF :] * 0.01]
    ).astype(np.float32)

    m = {
        "ctab": np.ascontiguousarray(ctab8).view(np.int16),
        "cidx": np.ascontiguousarray(cidx),
        "eidx": np.ascontiguousarray(_wrap16(ei.reshape(-1))),
        "w1p": w1p.reshape(128, -1).astype(FP8NP),
        "w2p": w2p.reshape(128, -1).astype(FP8NP),
        "wvp": wvp.reshape(128, -1).astype(ml_dtypes.bfloat16),
        "b1d": np.ascontiguousarray(4.0 * b1.reshape(4, 128).T),
        "b2d": np.ascontiguousarray(4.0 * b2.reshape(2, 128).T),
        "ebv": np.tile(np.exp(bv)[None, :], (BC, 1)).astype(np.float32),
        "ndsq": np.ascontiguousarray(-(dists * dists)),
        "featd": np.ascontiguousarray(feats),
        "wcr": np.tile(wc_r[None, :], (BC, 1)),
        "bcr": np.full((BC, 1), bc[0], np.float32),
    }
    return m


def _numpy_reference(inputs):
    """Pure-host fallback (unreachable for the spec's random fill)."""
    t = np.asarray(inputs["event_table"], np.float32)
    W1 = np.asarray(inputs["W1"], np.float32); b1 = np.asarray(inputs["b1"], np.float32)
    W2 = np.asarray(inputs["W2"], np.float32); b2 = np.asarray(inputs["b2"], np.float32)
    Wv = np.asarray(inputs["Wv"], np.float32); bv = np.asarray(inputs["bv"], np.float32)
    Wc = np.asarray(inputs["Wc"], np.float32); bc = np.asarray(inputs["bc"], np.float32)
    be = np.asarray(inputs["batch_event"], np.int64)
    bf = np.asarray(inputs["batch_features"], np.float32)
    bd = np.asarray(inputs["batch_distances"], np.float32)
    bx = np.asarray(inputs["batch_context"], np.int64)
    ee = t[be]                                    # (B, C, E)
    ce = t[bx]                                    # (B, N, C, E)
    pred = ee[:, 1, :]
    zv = pred @ Wv.T + bv
    var = np.log1p(np.exp(zv))
    de = np.exp(-(bd * bd) / var)
    ex = np.concatenate([de, bf], axis=1)

    def mlp(x):
        h = np.maximum(x @ W1.T + b1, 0.0)
        return np.maximum(h @ W2.T + b2, 0.0)

    er = mlp(ee.reshape(B, C * E))                # (B, H2)
    cr = mlp(ce.reshape(B * N, C * E)).reshape(B, N, H2)
    ern = er / np.maximum(np.linalg.norm(er, axis=-1, keepdims=True), 1e-12)
    crn = cr / np.maximum(np.linalg.norm(cr, axis=-1, keepdims=True), 1e-12)
    tr = np.einsum("bd,bnd->bn", ern, crn)        # (B, N)
    mus = np.array(MUS, np.float32)
    sig = np.array(SIGMAS, np.float32)
    kk = np.exp(-((tr[..., None] - mus) ** 2) / (2.0 * sig ** 2))
    pooled = kk.sum(axis=1)
    kp = np.log(np.clip(pooled, 1e-10, None)) * 0.01
    af = np.concatenate([ex, kp], axis=1)
    sc = af @ Wc[0] + bc[0]
    return (1.0 / (1.0 + np.exp(-sc)))[:, None].astype(np.float32)


def kernel(**inputs) -> np.ndarray:
    ctx = np.asarray(inputs["batch_context"], np.int64)
    ev = np.asarray(inputs["batch_event"], np.int64)
    for core in range(NCORES):
        sl = slice(core * BC, (core + 1) * BC)
        cp = ctx[sl].reshape(-1, 2, 2)
        ep = ev[sl].reshape(-1, 2, 2)
        keys = np.concatenate(
            [
                (cp[..., 0] * np.int64(V + 1) + cp[..., 1]).reshape(-1),
                (ep[..., 0] * np.int64(V + 1) + ep[..., 1]).reshape(-1),
            ]
        )
        if len(np.unique(keys)) > CT:
            return _numpy_reference(inputs)
    nc = _build_program(True)
    table8 = (np.asarray(inputs["event_table"], np.float32) * 8.0).astype(FP8NP)
    in_maps = [
        _prep_core_inputs(inputs, core, True, table8) for core in range(NCORES)
    ]
    res = run_bass_kernel_spmd(nc, in_maps, core_ids=list(range(NCORES)))
    return np.concatenate([r["out"] for r in res.results], axis=0)


if __name__ == "__main__":
    nc = _build_program(True)
    print("program built ok")


# revision 33
# speedup vs baseline: 1.5118x; 1.0666x over previous
"""Trainium2 Bass kernel for nn_EventPairCompositionModel (fp8 DoubleRow).

Strategy (data-parallel over batch, 8 cores, B=512 -> 64 per core):
  - Host builds a per-core compact table of COMPONENT-PAIR rows: the ~16K
    unique (idx[2c], idx[2c+1]) pairs each shard touches, stored as fp8e4m3
    (x8 scale) 600-elem rows padded to 768 bytes and DECLARED int16 so the
    SWDGE transpose-gather's 16-bit granularity lands fp8 element pairs
    (2u, 2u+1) on partition u%128 -- exactly the [K, 2, N] layout
    MatmulPerfMode.DoubleRow wants.  Pair rows mean 6 DoubleRow K-pairs per
    m-tile (vs 8 for single rows) and half the gather indices (the SWDGE
    ucode is per-index bound).
  - Context MLP (1200->512->256) entirely in fp8 DoubleRow (2 K-rows/cycle),
    processing 4 groups of 512 (b,n) pairs per weight pass so each stationary
    load amortizes over 4 matmuls; redundant LDWEIGHTS are removed by a
    post-compile pass.
  - The 64 event tuples ride pass 0 as a narrow (64-col) extra group.
  - Cosine numerators/|c|^2 via per-batch fp8 DR matmuls ([p,2,1]
    stationaries) landing on partition 0, staged to [1, 8192] rows and
    reshaped to [64 batch, 128 ctx] by one DMA; KNRM pooling on [64,128].
  - Distance-kernel path in bf16 off the event gather; final linear+sigmoid
    in [64 batch partitions, feature] layout.
All 8 cores run the identical program on their own batch shard (SPMD).
Host falls back to a numpy reference if a shard exceeds the int16 compact
table (cannot happen for the random fill).
"""

import numpy as np
import ml_dtypes

import concourse.bacc as bacc
import concourse.bass as bass
import concourse.tile as tile
import concourse.mybir as mybir
from concourse.bass_utils import run_bass_kernel_spmd
from concourse import library_config

F32 = mybir.dt.float32
BF16 = mybir.dt.bfloat16
FP8 = mybir.dt.float8e4
I16 = mybir.dt.int16
AF = mybir.ActivationFunctionType
DR = mybir.MatmulPerfMode.DoubleRow

# Problem shapes (hardcoded per spec)
B, N, C, E = 512, 128, 4, 300
V = 50000
H1, H2 = 512, 256
NF, NK = 8, 11
NCORES = 8
BC = B // NCORES          # 64 batches per core
CT = 32768                # compact pair-table rows (int16-indexable)
EP = 600                  # elems per pair row (2 components)
RU = 384                  # int16 units per table row (768 fp8 elems)
GROUPS = (BC * N) // 512  # 16 groups of 512 (b,n) pairs
NKP = 6                   # MLP1 K-pairs per m-tile (2 pair-comps x 3 slots)
FD = 28                   # features: 9 dist + 8 extracted + 11 kp
PASSES = 4                # groups per weight pass
NPASS = GROUPS // PASSES

MUS = [1.0, 0.9, 0.7, 0.5, 0.3, 0.1, -0.1, -0.3, -0.5, -0.7, -0.9]
SIGMAS = [1e-3] + [0.1] * 10

_PROGRAM_CACHE = {}


def _build_program(fast: bool = True):
    if True in _PROGRAM_CACHE:
        return _PROGRAM_CACHE[True]

    nc = bacc.Bacc("TRN2", target_bir_lowering=False, debug=False, num_swdge_queues=4)

    # ---- DRAM I/O ----
    ctab = nc.dram_tensor("ctab", (CT, RU), I16, kind="ExternalInput")
    cidx = nc.dram_tensor("cidx", (128, GROUPS * 2 * 32), I16, kind="ExternalInput")
    eidx = nc.dram_tensor("eidx", (128, 16), I16, kind="ExternalInput")
    w1p = nc.dram_tensor("w1p", (128, NKP * 4 * 256), FP8, kind="ExternalInput")
    w2p = nc.dram_tensor("w2p", (128, 2 * 2 * 256), FP8, kind="ExternalInput")
    wvp = nc.dram_tensor("wvp", (128, 4 * 9), BF16, kind="ExternalInput")
    b1d = nc.dram_tensor("b1d", (128, 4), F32, kind="ExternalInput")
    b2d = nc.dram_tensor("b2d", (128, 2), F32, kind="ExternalInput")
    ebv = nc.dram_tensor("ebv", (BC, 9), F32, kind="ExternalInput")
    ndsq = nc.dram_tensor("ndsq", (BC, 9), F32, kind="ExternalInput")
    featd = nc.dram_tensor("featd", (BC, NF), F32, kind="ExternalInput")
    wcr = nc.dram_tensor("wcr", (BC, FD), F32, kind="ExternalInput")
    bcr = nc.dram_tensor("bcr", (BC, 1), F32, kind="ExternalInput")
    out_d = nc.dram_tensor("out", (BC, 1), F32, kind="ExternalOutput")

    with tile.TileContext(nc) as tc:
        with (
            tc.tile_pool(name="consts", bufs=1) as cpool,
            tc.tile_pool(name="xt", bufs=2 * PASSES) as xtpool,
            tc.tile_pool(name="s1", bufs=PASSES + 2) as s1pool,
            tc.tile_pool(name="s2", bufs=PASSES + 2) as s2pool,
            tc.tile_pool(name="csq", bufs=PASSES + 2) as csqpool,
            tc.tile_pool(name="small", bufs=2) as smpool,
            tc.tile_pool(name="pm1", bufs=PASSES, space="PSUM") as pm1,
            tc.tile_pool(name="pm2", bufs=2, space="PSUM") as pm2,
            tc.tile_pool(name="pg", bufs=2, space="PSUM") as pgpool,
        ):
            nc.gpsimd.load_library(library_config.mlp)

            # ---- load constants ----
            cidx_s = cpool.tile([128, GROUPS * 2 * 32], I16)
            nc.sync.dma_start(cidx_s[:], cidx.ap())
            eidx_s = cpool.tile([128, 16], I16)
            nc.sync.dma_start(eidx_s[:], eidx.ap())
            w1p_s = cpool.tile([128, NKP * 4 * 256], FP8)
            nc.sync.dma_start(w1p_s[:], w1p.ap())
            w2p_s = cpool.tile([128, 2 * 2 * 256], FP8)
            nc.scalar.dma_start(w2p_s[:], w2p.ap())
            wvp_s = cpool.tile([128, 4 * 9], BF16)
            nc.scalar.dma_start(wvp_s[:], wvp.ap())
            b1_s = cpool.tile([128, 4], F32)
            nc.sync.dma_start(b1_s[:], b1d.ap())
            b2_s = cpool.tile([128, 2], F32)
            nc.sync.dma_start(b2_s[:], b2d.ap())
            ebv_s = cpool.tile([BC, 9], F32)
            nc.scalar.dma_start(ebv_s[:], ebv.ap())
            ndsq_s = cpool.tile([BC, 9], F32)
            nc.scalar.dma_start(ndsq_s[:], ndsq.ap())
            wcr_s = cpool.tile([BC, FD], F32)
            nc.scalar.dma_start(wcr_s[:], wcr.ap())
            bcr_s = cpool.tile([BC, 1], F32)
            nc.scalar.dma_start(bcr_s[:], bcr.ap())
            F_s = cpool.tile([BC, FD], F32)
            nc.sync.dma_start(F_s[:, 9 : 9 + NF], featd.ap())

            # ones pair for |c|^2 matmuls; K-group dim at stride 16 to satisfy
            # the dual-fp8 LDWEIGHTS AP restriction (pair step % 16 == 0)
            ones2c_s = cpool.tile([128, 32], FP8)
            nc.vector.memset(ones2c_s[:], 1.0)
            # batched KNRM constants: [64, k(11), 128] of -mu_k / -1/(2 sig_k^2)
            mub_s = cpool.tile([BC, NK * 128], F32)
            i2s_s = cpool.tile([BC, NK * 128], F32)
            for k in range(NK):
                nc.vector.memset(mub_s[:, 128 * k : 128 * (k + 1)], -MUS[k])
                nc.vector.memset(
                    i2s_s[:, 128 * k : 128 * (k + 1)],
                    -1.0 / (2.0 * SIGMAS[k] ** 2),
                )

            # persistent accumulators
            traw_s = cpool.tile([BC, 128], F32)        # 16*dot per (b, n)
            drow_s = cpool.tile([1, 512 * GROUPS], F32)  # dots, (g,s,n) on part 0
            nrow_s = cpool.tile([1, 512 * GROUPS], F32)  # |c|^2 likewise
            sgram_s = cpool.tile([128, 2 * BC], FP8)   # event reprs [p, m(2), 64]
            cse_s = cpool.tile([128, 2 * BC], FP8)     # their squares
            s1e_s = cpool.tile([128, 4 * BC], FP8)     # event s1 [p, mj(4), 64]
            predb_s = cpool.tile([128, 4 * BC], BF16)  # predicates [p, f(4), 64]
            ne2_s = cpool.tile([BC, 1], F32)
            ncsq0_s = cpool.tile([BC, 128], F32)
            trans_s = cpool.tile([BC, 128], F32)
            pooled_s = cpool.tile([BC, NK], F32)

            # ---- gathers (issued lazily so pool-slot reuse stays WAR-safe) ----
            xe_s = cpool.tile([128, 3 * 256], I16)   # event pair gather
            nc.gpsimd.dma_gather(
                out_ap=xe_s[:].rearrange("p (s i) -> p s i", s=3),
                in_ap=ctab.ap(),
                idxs_ap=eidx_s[:],
                num_idxs=256,
                num_idxs_reg=256,
                elem_size=RU,
                transpose=True,
            )
            xts = {}

            def issue_gather(g):
                # one 512-idx gather per component pair (512-idx bursts fit
                # the SWDGE descriptor-ring carveout; 2048 would deadlock it)
                if g >= GROUPS:
                    return
                xt = xtpool.tile([128, 2 * 3 * 512], I16, tag="xt", name=f"xt{g}")
                xv = xt[:].rearrange("p (cp s r) -> p cp s r", cp=2, s=3)
                for cp in range(2):
                    nc.gpsimd.dma_gather(
                        out_ap=xv[:, cp, :, :],
                        in_ap=ctab.ap(),
                        idxs_ap=cidx_s[:, 32 * (2 * g + cp) : 32 * (2 * g + cp + 1)],
                        num_idxs=512,
                        num_idxs_reg=512,
                        elem_size=RU,
                        transpose=True,
                    )
                xts[g] = xt

            for g in range(2 * PASSES):
                issue_gather(g)

            def ctx_rhs(g, cp, uj, w):
                # [p, 2(byte), w cols] fp8 view: group g, comp-pair cp, slot uj
                v = xts[g][:].bitcast(FP8).rearrange(
                    "p (cp s r i) -> p cp s i r", cp=2, s=3, i=2
                )
                return v[:, cp, uj, :, 0:w]

            def evt_rhs(cp, uj):
                v = xe_s[:].bitcast(FP8).rearrange("p (s r i) -> p s i r", s=3, i=2)
                return v[:, uj, :, 128 * cp : 128 * cp + BC]

            def w1_ap(kp, m):
                return w1p_s[:].rearrange(
                    "p (kp m i c) -> p kp m i c", kp=NKP, m=4, i=2
                )[:, kp, m, :, :]

            def w2_ap(q, m):
                return w2p_s[:].rearrange(
                    "p (q m i c) -> p q m i c", q=2, m=2, i=2
                )[:, q, m, :, :]

            # deferred per-batch dot/|c|^2 matmuls: flushed into the NEXT
            # pass's matmul stream so their dependency chains (act -> square)
            # never stall the PE at pass boundaries.  Outputs pack 4 streams
            # per PSUM bank at 32-aligned partitions.
            pending = []

            flush_n = [0]

            def flush_step(nstreams=2):
                # emit up to nstreams deferred dot/norm streams; called once
                # per m-slot of the following pass so the ring-2 PSUM recycle
                # always has a full m-window of slack
                sg_v = sgram_s[:].rearrange("p (m c) -> p m c", m=2)
                on_v = ones2c_s[:].rearrange("p (i x) -> p i x", i=2)[:, :, 0:1]
                for _ in range(nstreams):
                    if not pending:
                        return
                    kind, g, mv = pending.pop(0)
                    flush_n[0] += 1
                    PD = pgpool.tile([1, 512], F32, tag="pg", name=f"pd{flush_n[0]}")
                    mv_v = mv[:].rearrange("p (m x) -> p m x", m=2)
                    for s in range(4):
                        lane = 4 * g + s
                        nc.tensor.matmul(
                            PD[:, 128 * s : 128 * (s + 1)],
                            sg_v[:, :, lane : lane + 1] if kind == "d" else on_v,
                            mv_v[:, :, 128 * s : 128 * (s + 1)],
                            start=True, stop=True, perf_mode=DR,
                        )
                    nc.vector.tensor_copy(
                        out=(drow_s if kind == "d" else nrow_s)[
                            :, 512 * g : 512 * (g + 1)
                        ],
                        in_=PD[:],
                    )

            # ---- passes of PASSES context groups (+ events on pass 0) ----
            for pi in range(NPASS):
                grp = list(range(PASSES * pi, PASSES * (pi + 1)))
                with_evt = pi == 0
                # MLP1
                s1t = {}
                for g in grp:
                    s1t[g] = s1pool.tile([128, 4 * 512], FP8, tag="s1", name=f"s1_{g}")
                pl = {}
                for m in range(4):
                    for g in grp:
                        pl[g] = pm1.tile([128, 512], F32, tag="pm1", name=f"p1_{g}_{m}")
                    if with_evt:
                        pE = pm2.tile([128, BC], F32, tag="pm2", name=f"pe_{m}")
                    for kp in range(NKP):
                        cp, uj = kp // 3, kp % 3
                        st, sp = kp == 0, kp == NKP - 1
                        for g in grp:
                            nc.tensor.matmul(
                                pl[g][:], w1_ap(kp, m), ctx_rhs(g, cp, uj, 512),
                                start=st, stop=sp, perf_mode=DR,
                            )
                        if with_evt:
                            nc.tensor.matmul(
                                pE[:], w1_ap(kp, m), evt_rhs(cp, uj),
                                start=st, stop=sp, perf_mode=DR,
                            )
                    flush_step(2)   # previous pass's dots ride the m-slots
                    for g in grp:
                        nc.scalar.activation(
                            s1t[g][:, 512 * m : 512 * (m + 1)], pl[g][:],
                            AF.Relu, bias=b1_s[:, m : m + 1], scale=1.0 / 16,
                        )
                    if with_evt:
                        nc.scalar.activation(
                            s1e_s[:, BC * m : BC * (m + 1)], pE[:],
                            AF.Relu, bias=b1_s[:, m : m + 1], scale=1.0 / 16,
                        )

                # MLP2 in half-passes of 2 groups (PSUM budget)
                s2t = {}
                for g in grp:
                    s2t[g] = s2pool.tile([128, 2 * 512], FP8, tag="s2", name=f"s2_{g}")
                for half in range(PASSES // 2):
                    gh = grp[2 * half : 2 * half + 2]
                    for m in range(2):
                        p2 = {}
                        for g in gh:
                            p2[g] = pm2.tile(
                                [128, 512], F32, tag="pm2", name=f"p2_{g}_{m}"
                            )
                        if with_evt and half == 0:
                            pE2 = pm2.tile([128, BC], F32, tag="pm2", name=f"pe2_{m}")
                        for q in range(2):
                            st, sp = q == 0, q == 1
                            for g in gh:
                                nc.tensor.matmul(
                                    p2[g][:], w2_ap(q, m),
                                    s1t[g][:].rearrange("p (mj x) -> p mj x", mj=4)[
                                        :, 2 * q : 2 * q + 2, :
                                    ],
                                    start=st, stop=sp, perf_mode=DR,
                                )
                            if with_evt and half == 0:
                                nc.tensor.matmul(
                                    pE2[:], w2_ap(q, m),
                                    s1e_s[:].rearrange("p (mj x) -> p mj x", mj=4)[
                                        :, 2 * q : 2 * q + 2, :
                                    ],
                                    start=st, stop=sp, perf_mode=DR,
                                )
                        for g in gh:
                            nc.scalar.activation(
                                s2t[g][:, 512 * m : 512 * (m + 1)], p2[g][:],
                                AF.Relu, bias=b2_s[:, m : m + 1], scale=1.0 / 8,
                            )
                        if with_evt and half == 0:
                            nc.scalar.activation(
                                sgram_s[:, BC * m : BC * (m + 1)], pE2[:],
                                AF.Relu, bias=b2_s[:, m : m + 1], scale=1.0 / 8,
                            )

                    if with_evt and half == 0:
                        # event extras: squares, |e|^2, predicates, variances
                        nc.vector.tensor_mul(cse_s[:], sgram_s[:], sgram_s[:])
                        pne = pm2.tile([BC, 1], F32, tag="pm2", name="pne")
                        nc.tensor.matmul(
                            pne[:],
                            cse_s[:].rearrange("p (m c) -> p m c", m=2),
                            ones2c_s[:, 0:2].rearrange("p (o i) -> p i o", i=2),
                            start=True, stop=True, perf_mode=DR,
                        )
                        nc.scalar.copy(ne2_s[:], pne[:])

                        # predicates: elems 300..599 of the cp=0 pair rows
                        nc.scalar.copy(
                            predb_s[:].rearrange("p (s i l) -> p s i l", s=2, i=2),
                            xe_s[:].bitcast(FP8).rearrange(
                                "p (s r i) -> p s i r", s=3, i=2
                            )[:, 1:3, :, 0:BC],
                        )
                        pvar = pm2.tile([BC, 9], F32, tag="pm2", name="pvar")
                        for f in range(4):
                            nc.tensor.matmul(
                                pvar[:],
                                predb_s[:].rearrange("p (f l) -> p f l", f=4)[:, f, :],
                                wvp_s[:].rearrange("p (f v) -> p f v", f=4)[:, f, :],
                                start=(f == 0), stop=(f == 3),
                            )
                        ez = smpool.tile([BC, 9], F32, tag="sm9", name="ez")
                        nc.scalar.activation(ez[:], pvar[:], AF.Exp, scale=1.0 / 8)
                        ezb = smpool.tile([BC, 9], F32, tag="sm9", name="ezb")
                        nc.vector.tensor_mul(ezb[:], ez[:], ebv_s[:])
                        ez1 = smpool.tile([BC, 9], F32, tag="sm9", name="ez1")
                        nc.vector.tensor_scalar_add(ez1[:], ezb[:], 1.0)
                        var = smpool.tile([BC, 9], F32, tag="sm9", name="var")
                        nc.scalar.activation(var[:], ez1[:], AF.Ln)
                        rv = smpool.tile([BC, 9], F32, tag="sm9", name="rv")
                        nc.vector.reciprocal(rv[:], var[:])
                        qd = smpool.tile([BC, 9], F32, tag="sm9", name="qd")
                        nc.vector.tensor_mul(qd[:], ndsq_s[:], rv[:])
                        nc.scalar.activation(F_s[:, 0:9], qd[:], AF.Exp)

                    # squared activations now; dot/norm matmuls deferred
                    for g in gh:
                        csq = csqpool.tile(
                            [128, 2 * 512], FP8, tag="csq", name=f"csq_{g}"
                        )
                        nc.vector.tensor_mul(csq[:], s2t[g][:], s2t[g][:])
                        pending.append(("d", g, s2t[g]))
                        pending.append(("n", g, csq))
                # prefetch the gathers needed two passes ahead
                for g in grp:
                    issue_gather(g + 2 * PASSES)
            while pending:
                flush_step(2)

            # ---- tail: cosine, kernel pooling, final score ----
            nc.sync.dma_start(traw_s[:], drow_s[:])
            nc.sync.dma_start(ncsq0_s[:], nrow_s[:])
            prodn = smpool.tile([BC, 128], F32, tag="smT", name="prodn")
            nc.vector.tensor_tensor(
                out=prodn[:], in0=ncsq0_s[:],
                in1=ne2_s[:].broadcast_to([BC, 128]),
                op=mybir.AluOpType.mult,
            )
            prod1 = smpool.tile([BC, 128], F32, tag="smT", name="prod1")
            nc.vector.tensor_scalar_add(prod1[:], prodn[:], 1e-20)
            rec_s = smpool.tile([BC, 128], F32, tag="smT", name="rec")
            nc.vector.reciprocal(rec_s[:], prod1[:])
            nf_s = smpool.tile([BC, 128], F32, tag="smT", name="nf")
            nc.scalar.activation(nf_s[:], rec_s[:], AF.Sqrt)
            nc.vector.tensor_mul(trans_s[:], traw_s[:], nf_s[:])

            # batched KNRM pooling: all 11 kernels in [64, 11*128] ops
            dk = smpool.tile([BC, NK * 128], F32, tag="smB", name="dk")
            nc.vector.tensor_tensor(
                out=dk[:],
                in0=trans_s[:][:, None, :].broadcast_to([BC, NK, 128]),
                in1=mub_s[:].rearrange("b (k n) -> b k n", k=NK),
                op=mybir.AluOpType.add,
            )
            dsq = smpool.tile([BC, NK * 128], F32, tag="smB", name="dsq")
            nc.vector.tensor_mul(dsq[:], dk[:], dk[:])
            argb = smpool.tile([BC, NK * 128], F32, tag="smB", name="argb")
            nc.vector.tensor_mul(argb[:], dsq[:], i2s_s[:])
            argc = smpool.tile([BC, NK * 128], F32, tag="smB", name="argc")
            nc.vector.tensor_scalar_max(argc[:], argb[:], -87.0)
            ekb = smpool.tile([BC, NK * 128], F32, tag="smB", name="ekb")
            nc.scalar.activation(ekb[:], argc[:], AF.Exp)
            nc.vector.reduce_sum(
                out=pooled_s[:],
                in_=ekb[:].rearrange("b (k n) -> b k n", k=NK),
                axis=mybir.AxisListType.X,
            )

            poolc = smpool.tile([BC, NK], F32, tag="smK", name="poolc")
            nc.vector.tensor_scalar_max(poolc[:], pooled_s[:], 1e-10)
            nc.scalar.activation(F_s[:, 9 + NF :], poolc[:], AF.Ln)

            fw = smpool.tile([BC, FD], F32, tag="smK", name="fw")
            nc.vector.tensor_mul(fw[:], F_s[:], wcr_s[:])
            sc = smpool.tile([BC, 1], F32, tag="smS", name="sc")
            nc.vector.reduce_sum(out=sc[:], in_=fw[:], axis=mybir.AxisListType.X)
            sig = smpool.tile([BC, 1], F32, tag="smS", name="sig")
            nc.scalar.activation(sig[:], sc[:], AF.Sigmoid, bias=bcr_s[:])
            nc.sync.dma_start(out_d.ap(), sig[:])

    nc.compile()

    # Spread SWDGE gathers across the 4 queues (ucode locks each DMASW
    # semaphore lane to one queue; lanes are assigned round-robin in
    # scheduled order).
    import re as _re
    for blk in nc.m.functions[0].blocks:
        for inst in blk.instructions:
            if type(inst).__name__ == "InstDMAGatherAnt":
                for u in inst.sync_info.on_update:
                    m = _re.match(r"DMASW(\d+)_", u.ant_name or "")
                    if m:
                        inst.queue_num = int(m.group(1)) % 4
                        break

    _dedup_ldweights(nc)

    _PROGRAM_CACHE[True] = nc
    return nc


def _ldw_sig(inst):
    a = inst.ins[0]
    return (
        a.memref,
        a.offset,
        tuple(tuple(d) for d in a.ap),
        getattr(inst, "perf_mode", None),
        getattr(inst, "tile_position", None),
        getattr(inst, "tile_size", None),
        getattr(inst, "is_transpose", None),
    )


def _dedup_ldweights(nc):
    """Remove InstLdweights that reload the stationary operand already in the
    PE array.  The compile pass splits every matmul into LDWEIGHTS+MATMUL;
    back-to-back matmuls sharing weights then pay a redundant ~200ns load.
    Conservative: only drops loads carrying no semaphore waits/updates, so
    cross-engine ordering is untouched."""
    dropped = 0
    for blk in nc.m.functions[0].blocks:
        cur = None          # signature currently in the array
        keep = []
        for inst in blk.instructions:
            nm = type(inst).__name__
            if nm == "InstLdweights":
                sig = _ldw_sig(inst)
                si = inst.sync_info
                if sig == cur and (
                    si is None or (not si.on_wait and not si.on_update)
                ):
                    dropped += 1
                    continue
                cur = sig
            keep.append(inst)
        blk.instructions = keep
    return dropped


def _wrap16(flat_idx):
    """int16 index list -> (128, n/16) tile layout replicated into 8 stripes."""
    n = flat_idx.shape[0]
    t = np.zeros((16, n // 16), np.int16)
    t[np.arange(n) % 16, np.arange(n) // 16] = flat_idx
    return np.tile(t, (8, 1))


FP8NP = ml_dtypes.float8_e4m3fn


def _prep_core_inputs(inputs, core, fast=True, table8=None):
    """Host-side shard + weight re-layouts for one core."""
    W1 = np.asarray(inputs["W1"], np.float32)
    W2 = np.asarray(inputs["W2"], np.float32)
    Wv = np.asarray(inputs["Wv"], np.float32)
    Wc = np.asarray(inputs["Wc"], np.float32)
    b1 = np.asarray(inputs["b1"], np.float32)
    b2 = np.asarray(inputs["b2"], np.float32)
    bv = np.asarray(inputs["bv"], np.float32)
    bc = np.asarray(inputs["bc"], np.float32)

    sl = slice(core * BC, (core + 1) * BC)
    ev = np.asarray(inputs["batch_event"][sl], np.int64)          # (BC, C)
    feats = np.asarray(inputs["batch_features"][sl], np.float32)  # (BC, NF)
    dists = np.asarray(inputs["batch_distances"][sl], np.float32) # (BC, 9)
    ctx = np.asarray(inputs["batch_context"][sl], np.int64)       # (BC, N, C)

    if table8 is None:
        table8 = (np.asarray(inputs["event_table"], np.float32) * 8.0).astype(FP8NP)

    # component-pair keys: (idx0, idx1) and (idx2, idx3) per (b, n) / event
    ctxp = ctx.reshape(BC, N, 2, 2)          # (b, n, cp, which)
    evp = ev.reshape(BC, 2, 2)
    keys = np.concatenate(
        [
            (ctxp[..., 0] * np.int64(V + 1) + ctxp[..., 1]).reshape(-1),
            (evp[..., 0] * np.int64(V + 1) + evp[..., 1]).reshape(-1),
        ]
    )
    uniq, inv = np.unique(keys, return_inverse=True)
    assert len(uniq) <= CT
    u0 = (uniq // (V + 1)).astype(np.int64)
    u1 = (uniq % (V + 1)).astype(np.int64)
    ctab8 = np.zeros((CT, 2 * RU), FP8NP)
    ctab8[: len(uniq), 0:E] = table8[u0]
    ctab8[: len(uniq), E : 2 * E] = table8[u1]
    nctx = BC * N * 2
    rctx = inv[:nctx].astype(np.int16).reshape(BC, N, 2)
    rev = inv[nctx:].astype(np.int16).reshape(BC, 2)

    # context gathers: per (group g, comp-pair cp), 512 idxs ordered (s, n)
    ci = rctx.reshape(GROUPS, 4, N, 2).transpose(0, 3, 1, 2)  # g, cp, s, n
    cidx = np.concatenate(
        [
            _wrap16(ci[g, cp].reshape(-1))
            for g in range(GROUPS)
            for cp in range(2)
        ],
        axis=1,
    )
    # event gather: 256 idxs, j = cp*128 + lane; lanes >= BC gather row 0
    ei = np.zeros((2, 128), np.int16)
    ei[:, :BC] = rev.T

    # W1 packed for DoubleRow: [p, kp(cp,uj), m, i, mcol]
    W1x = (8.0 * W1).astype(np.float32)          # (H1, C*E)
    W2x = (8.0 * W2).astype(np.float32)          # (H2, H1)
    p_i = np.arange(128)
    w1p = np.zeros((128, NKP, 4, 2, 128), np.float32)
    for cp in range(2):
        for uj in range(3):
            # pair-row element index e in [0, 768); maps to W1 column
            e = 256 * uj + 2 * p_i[:, None] + np.arange(2)[None, :]  # (128, 2)
            comp = 2 * cp + (e >= E)
            off = e - E * (e >= E)
            valid = e < EP
            col = np.minimum(comp * E + off, C * E - 1)
            src = W1x[:, col] * valid[None, :, :]   # (H1, 128, 2)
            blk = src.reshape(4, 128, 128, 2).transpose(2, 0, 3, 1)
            w1p[:, 3 * cp + uj] = blk
    w2p = np.zeros((128, 2, 2, 2, 128), np.float32)
    for q in range(2):
        for i in range(2):
            src = W2x[:, 128 * (2 * q + i) + p_i]      # (H2, 128)
            w2p[:, q, :, i, :] = src.reshape(2, 128, 128).transpose(2, 0, 1)
    # predicates live at pair-row elems 300..599 (comp 1 of the cp=0 row):
    # f slots are (uj, i) for uj in {1, 2}
    wvp = np.zeros((128, 4, 9), np.float32)
    for f in range(4):
        e = 256 * (1 + f // 2) + 2 * p_i + (f % 2)
        k = e - E
        valid = (k >= 0) & (k < E)
        wvp[:, f, :] = Wv[:, np.clip(k, 0, E - 1)].T * valid[:, None]

    wc_r = np.concatenate(
        [Wc[0, 0:9], Wc[0, 9 : 9 + NF], Wc[0, 9 + NF :] * 0.01]
    ).astype(np.float32)

    m = {
        "ctab": np.ascontiguousarray(ctab8).view(np.int16),
        "cidx": np.ascontiguousarray(cidx),
        "eidx": np.ascontiguousarray(_wrap16(ei.reshape(-1))),
        "w1p": w1p.reshape(128, -1).astype(FP8NP),
        "w2p": w2p.reshape(128, -1).astype(FP8NP),
        "wvp": wvp.reshape(128, -1).astype(ml_dtypes.bfloat16),
        "b1d": np.ascontiguousarray(4.0 * b1.reshape(4, 128).T),
        "b2d": np.ascontiguousarray(4.0 * b2.reshape(2, 128).T),
        "ebv": np.tile(np.exp(bv)[None, :], (BC, 1)).astype(np.float32),
        "ndsq": np.ascontiguousarray(-(dists * dists)),
        "featd": np.ascontiguousarray(feats),
        "wcr": np.tile(wc_r[None, :], (BC, 1)),
        "bcr": np.full((BC, 1), bc[0], np.float32),
    }
    return m


def _numpy_reference(inputs):
    """Pure-host fallback (unreachable for the spec's random fill)."""
    t = np.asarray(inputs["event_table"], np.float32)
    W1 = np.asarray(inputs["W1"], np.float32); b1 = np.asarray(inputs["b1"], np.float32)
    W2 = np.asarray(inputs["W2"], np.float32); b2 = np.asarray(inputs["b2"], np.float32)
    Wv = np.asarray(inputs["Wv"], np.float32); bv = np.asarray(inputs["bv"], np.float32)
    Wc = np.asarray(inputs["Wc"], np.float32); bc = np.asarray(inputs["bc"], np.float32)
    be = np.asarray(inputs["batch_event"], np.int64)
    bf = np.asarray(inputs["batch_features"], np.float32)
    bd = np.asarray(inputs["batch_distances"], np.float32)
    bx = np.asarray(inputs["batch_context"], np.int64)
    ee = t[be]                                    # (B, C, E)
    ce = t[bx]                                    # (B, N, C, E)
    pred = ee[:, 1, :]
    zv = pred @ Wv.T + bv
    var = np.log1p(np.exp(zv))
    de = np.exp(-(bd * bd) / var)
    ex = np.concatenate([de, bf], axis=1)

    def mlp(x):
        h = np.maximum(x @ W1.T + b1, 0.0)
        return np.maximum(h @ W2.T + b2, 0.0)

    er = mlp(ee.reshape(B, C * E))                # (B, H2)
    cr = mlp(ce.reshape(B * N, C * E)).reshape(B, N, H2)
    ern = er / np.maximum(np.linalg.norm(er, axis=-1, keepdims=True), 1e-12)
    crn = cr / np.maximum(np.linalg.norm(cr, axis=-1, keepdims=True), 1e-12)
    tr = np.einsum("bd,bnd->bn", ern, crn)        # (B, N)
    mus = np.array(MUS, np.float32)
    sig = np.array(SIGMAS, np.float32)
    kk = np.exp(-((tr[..., None] - mus) ** 2) / (2.0 * sig ** 2))
    pooled = kk.sum(axis=1)
    kp = np.log(np.clip(pooled, 1e-10, None)) * 0.01
    af = np.concatenate([ex, kp], axis=1)
    sc = af @ Wc[0] + bc[0]
    return (1.0 / (1.0 + np.exp(-sc)))[:, None].astype(np.float32)


def kernel(**inputs) -> np.ndarray:
    ctx = np.asarray(inputs["batch_context"], np.int64)
    ev = np.asarray(inputs["batch_event"], np.int64)
    for core in range(NCORES):
        sl = slice(core * BC, (core + 1) * BC)
        cp = ctx[sl].reshape(-1, 2, 2)
        ep = ev[sl].reshape(-1, 2, 2)
        keys = np.concatenate(
            [
                (cp[..., 0] * np.int64(V + 1) + cp[..., 1]).reshape(-1),
                (ep[..., 0] * np.int64(V + 1) + ep[..., 1]).reshape(-1),
            ]
        )
        if len(np.unique(keys)) > CT:
            return _numpy_reference(inputs)
    nc = _build_program(True)
    table8 = (np.asarray(inputs["event_table"], np.float32) * 8.0).astype(FP8NP)
    in_maps = [
        _prep_core_inputs(inputs, core, True, table8) for core in range(NCORES)
    ]
    res = run_bass_kernel_spmd(nc, in_maps, core_ids=list(range(NCORES)))
    return np.concatenate([r["out"] for r in res.results], axis=0)


if __name__ == "__main__":
    nc = _build_program(True)
    print("program built ok")
